# revision 16
# baseline (speedup 1.0000x reference)
"""Trainium2 Bass kernel for DFlashAttentionV5.

Reference computation (fp32, single device):
    Q/K/V/Kctx/Vctx projections -> rmsnorm(Q), rmsnorm(K_full) -> softmax
    attention over concat(ctx, self) keys/values -> output projection.

Sharding over 8 NeuronCores: batch (2-way) x head-group (4-way).
Core c handles batch b = c // 4 and heads 4*g..4*g+3 where g = c % 4.
Each core computes attention for its 4 local heads, then the transposed
full-width output-projection partial outT[2048 cols, 1024 tokens] (same
PE cost as a 512-col slice: the contraction shrinks to the 512 local E
dims), and a single ReduceScatter sums the partials across the 4-core
batch group, leaving each core its 512-row slice of outT. The collective
cost model charges ~15us fixed + out_bytes/40GBps per op, so one 1MB-out
ReduceScatter (~41us) beats four chained 1MB-out AllGathers (~165us).

All matmuls run in bf16 (fp32 PSUM accumulation); softmax statistics and
normalization factors are computed in fp32. x/ctx arrive pre-transposed
from the host ([D, tokens]) so no on-device transposes are needed.

Self-contained: hardcodes all shapes; only imports concourse + numpy.
"""

import math

import numpy as np
import ml_dtypes

import concourse.bass as bass
import concourse.mybir as mybir
import concourse.tile as tile
from concourse.bass_utils import run_bass_kernel_spmd

BF16 = mybir.dt.bfloat16
F32 = mybir.dt.float32
AF = mybir.ActivationFunctionType
ALU = mybir.AluOpType

# Problem dims
B, K, CTX, D, H, HD = 2, 1024, 2048, 2048, 16, 128
S = CTX + K            # 3072 keys per query
NCORES = 8
GROUPS = 4             # head groups (tensor-parallel within a batch)
NH = H // GROUPS       # 4 local heads per core
E = H * HD             # 2048
EW = NH * HD           # 512 local attention width / weight shard width
DCH = D // 128         # 16 contraction chunks
SCH = S // 128         # 24 key chunks
TCH = K // 128         # 8 query-token chunks
SCALE = 1.0 / math.sqrt(HD)
EPS = 1e-6
REPLICA_GROUPS = [[0, 1, 2, 3], [4, 5, 6, 7]]

_CACHE = {}


def _build(with_mask: bool):
    """Build the SPMD bass program (same program on all 8 cores)."""
    nc = bass.Bass(num_devices=NCORES)

    xT_d = nc.declare_dram_parameter("xT", [D, K], BF16, isOutput=False)
    cT_d = nc.declare_dram_parameter("cT", [D, CTX], BF16, isOutput=False)
    wq_d = nc.declare_dram_parameter("wq", [D, EW], BF16, isOutput=False)
    wk_d = nc.declare_dram_parameter("wk", [D, EW], BF16, isOutput=False)
    wv_d = nc.declare_dram_parameter("wv", [D, EW], BF16, isOutput=False)
    wck_d = nc.declare_dram_parameter("wck", [D, EW], BF16, isOutput=False)
    wcv_d = nc.declare_dram_parameter("wcv", [D, EW], BF16, isOutput=False)
    wo_d = nc.declare_dram_parameter("wo", [EW, E], BF16, isOutput=False)
    qnw_d = nc.declare_dram_parameter("qnw", [HD, 1], F32, isOutput=False)
    knw_d = nc.declare_dram_parameter("knw", [HD, 1], F32, isOutput=False)
    if with_mask:
        mt_d = nc.declare_dram_parameter("maskT", [S, K], F32, isOutput=False)
    # transposed output slice [out-cols 512, tokens 1024], host transposes
    out_d = nc.declare_dram_parameter("out", [EW, K], BF16, isOutput=True)

    with tile.TileContext(nc, num_cores=NCORES) as tc:
        with (
            tc.tile_pool(name="const", bufs=1) as constp,
            tc.tile_pool(name="perm", bufs=1) as perm,
            tc.tile_pool(name="stat", bufs=2) as statp,
            tc.tile_pool(name="bc", bufs=2) as bcp,
            tc.tile_pool(name="psA", bufs=3, space="PSUM") as psA,
            tc.tile_pool(name="ps1", bufs=2, space="PSUM") as ps1,
            tc.tile_pool(name="dram", bufs=1, space="DRAM") as dram,
        ):
            ones_col = constp.tile([128, 1], BF16)
            nc.any.memset(ones_col, 1.0)
            ones_row = constp.tile([1, 128], BF16)
            nc.any.memset(ones_row, 1.0)
            qnw_sb = constp.tile([HD, 1], F32)
            knw_sb = constp.tile([HD, 1], F32)

            # Resident tensors (bf16):
            #   K_sb[h]  [128=hd, 3072=s] per local head (ctx keys then self)
            #   V_sb[s]  [128=s-chunk, 512=4 heads x hd], s 0..15 ctx, 16..23 self
            #   QT_sb[h] [128=hd, 1024=q]
            K_sb = [perm.tile([128, S], BF16, tag=f"K{h}", bufs=1, name=f"K{h}")
                    for h in range(NH)]
            V_sb = [perm.tile([128, EW], BF16, tag=f"V{s}", bufs=1, name=f"V{s}")
                    for s in range(SCH)]
            QT_sb = [perm.tile([128, K], BF16, tag=f"Q{h}", bufs=1, name=f"Q{h}")
                     for h in range(NH)]
            attnT_sb = [perm.tile([128, K], BF16, tag=f"A{h}", bufs=1,
                                  name=f"A{h}") for h in range(NH)]

            # full-width transposed out-proj partials, reduce-scattered
            # per query-token half so RS(half0) overlaps half1 compute
            QW = K // 2
            partial_d = [dram.tile([E, QW], BF16, name=f"part{i}")
                         for i in range(2)]
            rs_out_d = [dram.tile([EW, QW], BF16, name=f"rsout{i}")
                        for i in range(2)]

            # ---- helper: rmsnorm in transposed layout.
            # ps [128=hd, width=tokens] fp32 PSUM -> dest bf16 SBUF.
            # norm over hd (partitions): mean of squares via ones-matmul,
            # rsqrt via reciprocal+sqrt, broadcast across partitions via K=1
            # matmul, apply with one scalar_tensor_tensor (folds norm weight).
            def rms_norm_T(sqp, ps, dest_ap, width, nw_sb):
                # norm over hd (partitions): mean of squares via ones-matmul,
                # rsqrt via reciprocal+sqrt, broadcast across partitions via
                # K=1 matmul, applied with one scalar_tensor_tensor (also
                # folds the norm weight).
                sqt = sqp.tile([128, 1024], BF16, tag="sq")
                nc.scalar.square(sqt[:, :width], ps[:, :width])
                for j in range(width // 512):
                    js = slice(j * 512, (j + 1) * 512)
                    ps_s = ps1.tile([128, 512], F32, tag="ps1")
                    nc.tensor.matmul(ps_s[0:1, :], ones_col[:], sqt[:, js],
                                     start=True, stop=True)
                    mean = statp.tile([1, 512], F32, tag="mean")
                    nc.vector.tensor_scalar(mean[:], ps_s[0:1, :], 1.0 / HD, EPS,
                                            ALU.mult, ALU.add)
                    rec = statp.tile([1, 512], F32, tag="rec")
                    nc.vector.reciprocal(rec[:], mean[:])
                    rs = statp.tile([1, 512], BF16, tag="rs")
                    nc.scalar.sqrt(rs[:], rec[:])  # rsqrt = sqrt(1/x), bf16
                    ps_b = ps1.tile([128, 512], F32, tag="ps1")
                    nc.tensor.matmul(ps_b[:], ones_row[:], rs[:],
                                     start=True, stop=True)
                    bc = bcp.tile([128, 512], F32, tag="bc")
                    nc.scalar.copy(bc[:], ps_b[:])
                    nc.vector.scalar_tensor_tensor(
                        dest_ap[:, js], ps[:, js], nw_sb[:], bc[:],
                        ALU.mult, ALU.mult)

            # ================= projection phase =================
            with (
                tc.tile_pool(name="srcT", bufs=1) as srcTp,
                tc.tile_pool(name="wstream", bufs=4) as wstream,
                tc.tile_pool(name="wwide", bufs=1) as wwide,
                tc.tile_pool(name="sqp", bufs=2) as sqp,
            ):
                # d-chunk accessor over grouped source tiles [128, 4*1024]
                def src_at(grp, d):
                    return grp[d // 4], (d % 4) * 1024

                def load_set(grp, dram_ap, split_first=False):
                    # dram_ap: [D, 1024] (d-major); one DMA per 4 d-chunks.
                    # split_first peels d-chunk 0 into its own small DMA so
                    # the first dependent matmul can start sooner.
                    if split_first:
                        nc.sync.dma_start(
                            grp[0][:, 0:1024],
                            dram_ap[0:128, :])
                        nc.sync.dma_start(
                            grp[0][:, 1024:4096].rearrange("p (a t) -> p a t",
                                                           t=1024),
                            dram_ap[128:512, :]
                            .rearrange("(a p) t -> p a t", p=128))
                    else:
                        nc.sync.dma_start(
                            grp[0][:].rearrange("p (a t) -> p a t", t=1024),
                            dram_ap[0:512, :]
                            .rearrange("(a p) t -> p a t", p=128))
                    for i in range(1, 4):
                        nc.sync.dma_start(
                            grp[i][:].rearrange("p (a t) -> p a t", t=1024),
                            dram_ap[i * 512:(i + 1) * 512, :]
                            .rearrange("(a p) t -> p a t", p=128))

                # Q^T / K^T projections (weight-stationary):
                # psum[c] [128=col-chunk(head), 1024 tokens] += w[d,c].T @ srcT[d]
                def load_wchunk(w_d, c, name=None):
                    wch = wstream.tile([128, D], BF16, tag="w", name=name)
                    nc.sync.dma_start(
                        wch[:].rearrange("p (a q) -> p a q", q=128),
                        w_d[:, c * 128:(c + 1) * 128]
                        .rearrange("(a p) q -> p a q", p=128))
                    return wch

                def qk_proj(w_d, srcT, dest_of_chunk, nw_sb, pre=None):
                    for c in range(EW // 128):
                        wch = pre if (c == 0 and pre is not None) \
                            else load_wchunk(w_d, c)
                        ps = psA.tile([128, 1024], F32, tag="psA",
                                      name=f"psqk{c}")
                        for d in range(DCH):
                            st, off = src_at(srcT, d)
                            for j in range(2):
                                nc.tensor.matmul(
                                    ps[:, j * 512:(j + 1) * 512],
                                    wch[:, d * 128:(d + 1) * 128],
                                    st[:, off + j * 512:off + j * 512 + 512],
                                    start=(d == 0), stop=(d == DCH - 1))
                        dest, off = dest_of_chunk(c)
                        rms_norm_T(sqp, ps, dest[:, off:off + 1024], 1024, nw_sb)

                # V projections (activation-stationary):
                # V_sb[s] [128=tokens, 512=cols] += srcT[d][:,t-chunk].T @ wv[d]
                def load_wide(w_d, tag):
                    grp = []
                    for i in range(4):
                        wt = wwide.tile([128, 4 * EW], BF16, tag=f"wv{i}",
                                        bufs=1, name=f"wv{i}_{tag}")
                        nc.sync.dma_start(
                            wt[:].rearrange("p (a q) -> p a q", q=EW),
                            w_d[i * 512:(i + 1) * 512, :]
                            .rearrange("(a p) q -> p a q", p=128))
                        grp.append(wt)
                    return grp

                def v_proj(wv_grp, srcT, s_base):
                    for t in range(TCH):
                        ps = ps1.tile([128, 512], F32, tag="ps1")
                        for d in range(DCH):
                            st, off = src_at(srcT, d)
                            wvt = wv_grp[d // 4]
                            wo_off = (d % 4) * EW
                            nc.tensor.matmul(
                                ps[:], st[:, off + t * 128:off + (t + 1) * 128],
                                wvt[:, wo_off:wo_off + EW],
                                start=(d == 0), stop=(d == DCH - 1))
                        nc.vector.tensor_copy(V_sb[s_base + t][:], ps[:])

                # slot sets: A = xT then ctx-half1 (recycled), B = ctx-half0
                setA = [srcTp.tile([128, 4096], BF16, tag=f"sa{i}", bufs=1,
                                   name=f"xT{i}") for i in range(4)]
                setB = [srcTp.tile([128, 4096], BF16, tag=f"sb{i}", bufs=1,
                                   name=f"cTa{i}") for i in range(4)]
                pre_wq = load_wchunk(wq_d, 0, name="prewq")
                load_set(setA, xT_d, split_first=True)
                nc.sync.dma_start(qnw_sb[:], qnw_d[:])
                nc.sync.dma_start(knw_sb[:], knw_d[:])

                # self tokens (block A): Q, K_self, V_self
                qk_proj(wq_d, setA, lambda c: (QT_sb[c], 0), qnw_sb, pre=pre_wq)
                load_set(setB, cT_d[:, 0:1024])
                wv_sb = load_wide(wv_d, "s")
                qk_proj(wk_d, setA, lambda c: (K_sb[c], CTX), knw_sb)
                v_proj(wv_sb, setA, CTX // 128)

                # ctx half 0 (block B): K_ctx[:, 0:1024], V_ctx s-chunks 0..7
                wcv_sb = load_wide(wcv_d, "c0")
                qk_proj(wck_d, setB, lambda c: (K_sb[c], 0), knw_sb)
                v_proj(wcv_sb, setB, 0)

                # ctx half 1 reuses set A slots
                setC = [srcTp.tile([128, 4096], BF16, tag=f"sa{i}", bufs=1,
                                   name=f"cTb{i}") for i in range(4)]
                load_set(setC, cT_d[:, 1024:2048])
                wcv2_sb = load_wide(wcv_d, "c1")
                qk_proj(wck_d, setC, lambda c: (K_sb[c], 1024), knw_sb)
                v_proj(wcv2_sb, setC, TCH)

            # ================= attention + output phase =================
            with (
                tc.tile_pool(name="probsT", bufs=8) as probsp,
                tc.tile_pool(name="dacc", bufs=2) as daccp,
                tc.tile_pool(name="accb", bufs=2) as accbp,
                tc.tile_pool(name="wop", bufs=1) as wop,
                tc.tile_pool(name="ostg", bufs=3) as ostgp,
                tc.tile_pool(name="mrow", bufs=4) as mrowp,
            ):
                # w_out rows for local head h (global rows (g*NH+h)*128..),
                # all 2048 columns: [128=hd, 2048=out-cols]
                wo_sb = [wop.tile([128, E], BF16, tag=f"wo{h}", bufs=1,
                                  name=f"wo{h}") for h in range(NH)]
                for h in range(NH):
                    nc.sync.dma_start(wo_sb[h][:],
                                      wo_d[h * 128:(h + 1) * 128, :])

                # attention for one local head, transposed scores:
                # scoresT[s-chunk] [128=s, 1024=q] = K_chunk @ Q^T  (no max
                # subtraction: scores ~ N(0,1) after rmsnorm + 1/sqrt(HD))
                def attention(h, q0=0, qw=K):
                    # processes queries [q0, q0+qw) for local head h
                    nj = qw // 512
                    ps_pv = psA.tile([128, 1024], F32, tag="psA", name=f"pv{h}_{q0}")
                    acc = daccp.tile([128, 1024], F32, tag="dacc", name=f"dac{h}")
                    for s in range(SCH):
                        ps_sT = psA.tile([128, 1024], F32, tag="psA",
                                         name=f"sT{h}_{q0}_{s}")
                        for j in range(nj):
                            nc.tensor.matmul(
                                ps_sT[:, j * 512:(j + 1) * 512],
                                K_sb[h][:, s * 128:(s + 1) * 128],
                                QT_sb[h][:, q0 + j * 512:q0 + (j + 1) * 512],
                                start=True, stop=True)
                        if with_mask:
                            mrow = mrowp.tile([128, K], F32, tag="mrow")
                            nc.sync.dma_start(
                                mrow[:, :qw],
                                mt_d[s * 128:(s + 1) * 128, q0:q0 + qw])
                            nc.vector.tensor_tensor(ps_sT[:, :qw], ps_sT[:, :qw],
                                                    mrow[:, :qw], ALU.add)
                        pT = probsp.tile([128, 1024], BF16, tag="pT")
                        nc.scalar.activation(pT[:, :qw], ps_sT[:, :qw], AF.Exp,
                                             scale=SCALE)
                        first, last = (s == 0), (s == SCH - 1)
                        # probs accumulate on the DVE (f32) for the softmax
                        # denominators; the PE only does scores and PV
                        if first:
                            nc.vector.tensor_copy(acc[:, :qw], pT[:, :qw])
                        else:
                            nc.vector.tensor_tensor(acc[:, :qw], acc[:, :qw],
                                                    pT[:, :qw], ALU.add)
                        for j in range(nj):
                            js = slice(j * 512, (j + 1) * 512)
                            nc.tensor.matmul(
                                ps_pv[:, js],
                                V_sb[s][:, h * 128:(h + 1) * 128], pT[:, js],
                                start=first, stop=last)
                    # normalize: attnT = ps_pv * (1/denom), denom broadcast
                    # across partitions via K=1 matmul; writes the resident
                    # attnT_sb[h] columns [q0, q0+qw)
                    accb = accbp.tile([128, 1024], BF16, tag="accb")
                    nc.vector.tensor_copy(accb[:, :qw], acc[:, :qw])
                    for j in range(nj):
                        js = slice(j * 512, (j + 1) * 512)
                        ps_d = ps1.tile([128, 512], F32, tag="ps1")
                        nc.tensor.matmul(ps_d[0:1, :], ones_col[:], accb[:, js],
                                         start=True, stop=True)
                        rec = statp.tile([1, 512], F32, tag="rec")
                        nc.vector.reciprocal(rec[:], ps_d[0:1, :])
                        rb = statp.tile([1, 512], BF16, tag="rb")
                        nc.vector.tensor_copy(rb[:], rec[:])
                        ps_b = ps1.tile([128, 512], F32, tag="ps1")
                        nc.tensor.matmul(ps_b[:], ones_row[:], rb[:],
                                         start=True, stop=True)
                        bc = bcp.tile([128, 512], F32, tag="bc")
                        nc.scalar.copy(bc[:], ps_b[:])
                        nc.vector.tensor_tensor(
                            attnT_sb[h][:, q0 + j * 512:q0 + (j + 1) * 512],
                            ps_pv[:, js], bc[:], ALU.mult)

                # two-stage pipeline over query-token halves: attention for
                # all 4 heads on half q-tokens, then the transposed out-proj
                # partial for that half (outT[c][128 cols, 512 tok] = sum_h
                # wo_sb[h][:,c].T @ attnT[h], PSUM-accumulated over the 4
                # local heads), staged to bf16 and DMAd to DRAM; the
                # ReduceScatter for half 0 then overlaps all of half 1's
                # compute, leaving only RS(half1) (~28us) on the tail.
                for half in range(2):
                    q0 = half * QW
                    for h in range(NH):
                        attention(h, q0=q0, qw=QW)
                    for c in range(E // 128):
                        ps = ps1.tile([128, 512], F32, tag="ps1",
                                      name=f"ot{half}_{c}")
                        for h in range(NH):
                            nc.tensor.matmul(
                                ps[:], wo_sb[h][:, c * 128:(c + 1) * 128],
                                attnT_sb[h][:, q0:q0 + QW],
                                start=(h == 0), stop=(h == NH - 1))
                        stg = ostgp.tile([128, QW], BF16, tag="ostg")
                        nc.vector.tensor_copy(stg[:], ps[:])
                        nc.sync.dma_start(
                            partial_d[half][c * 128:(c + 1) * 128, :], stg[:])

                    nc.gpsimd.collective_compute(
                        "ReduceScatter", ALU.add,
                        replica_groups=REPLICA_GROUPS,
                        ins=[partial_d[half][:].opt()],
                        outs=[rs_out_d[half][:].opt()],
                    )
                    nc.sync.dma_start(out_d[:, q0:q0 + QW], rs_out_d[half][:])

    return nc


def _split_multiwaits(nc):
    """walrus codegen in this container rejects instructions with more than
    one semaphore wait; split the excess onto preceding NoOps on the same
    engine."""
    for f in nc.m.functions:
        for blk in f.blocks:
            idx = 0
            while idx < len(blk.instructions):
                inst = blk.instructions[idx]
                si = inst.sync_info
                maxw = 1
                if si is None or len(si.on_wait) <= maxw:
                    idx += 1
                    continue
                waits = list(si.on_wait)
                ncarry = (len(waits) - 1) // maxw  # leave <=maxw on inst
                for k in range(ncarry):
                    chunk = waits[k * maxw:(k + 1) * maxw]
                    nop = mybir.InstNoOp(
                        name=nc.get_next_instruction_name(),
                        ins=[], outs=[],
                        bass_nofuse=True,
                        sync_info=mybir.SyncInfo(on_wait=chunk, on_update=[]),
                    )
                    nop.engine = inst.engine
                    nc.register_instruction(nop)
                    blk.instructions.insert(idx, nop)
                    idx += 1
                si.on_wait = waits[ncarry * maxw:]
                idx += 1


def _get_program(with_mask: bool):
    key = ("prog", with_mask)
    if key not in _CACHE:
        nc = _build(with_mask)
        _split_multiwaits(nc)
        _CACHE[key] = nc
    return _CACHE[key]


def kernel(x, context, attn_mask, w_q, w_k, w_v, w_ctx_k, w_ctx_v, w_out,
           q_norm_w, k_norm_w):
    x = np.asarray(x, np.float32)
    context = np.asarray(context, np.float32)
    attn_mask = np.asarray(attn_mask, np.float32)
    w_q = np.asarray(w_q, np.float32)
    w_k = np.asarray(w_k, np.float32)
    w_v = np.asarray(w_v, np.float32)
    w_ctx_k = np.asarray(w_ctx_k, np.float32)
    w_ctx_v = np.asarray(w_ctx_v, np.float32)
    w_out = np.asarray(w_out, np.float32)
    q_norm_w = np.asarray(q_norm_w, np.float32)
    k_norm_w = np.asarray(k_norm_w, np.float32)

    with_mask = bool(np.any(attn_mask))
    nc = _get_program(with_mask)
    in_maps = _prepare_in_maps(x, context, attn_mask, w_q, w_k, w_v, w_ctx_k,
                               w_ctx_v, w_out, q_norm_w, k_norm_w, with_mask)

    res = run_bass_kernel_spmd(nc, in_maps, list(range(NCORES))).results
    return _assemble(res)


def _assemble(res):
    out = np.empty((B, K, D), np.float32)
    for c in range(NCORES):
        b, g = c // GROUPS, c % GROUPS
        out[b, :, g * EW:(g + 1) * EW] = res[c]["out"].astype(np.float32).T
    return out


def _prepare_in_maps(x, context, attn_mask, w_q, w_k, w_v, w_ctx_k, w_ctx_v,
                     w_out, q_norm_w, k_norm_w, with_mask):
    bf16 = ml_dtypes.bfloat16
    xT = [np.ascontiguousarray(x[b].T).astype(bf16) for b in range(B)]
    cT = [np.ascontiguousarray(context[b].T).astype(bf16) for b in range(B)]
    in_maps = []
    for c in range(NCORES):
        b, g = c // GROUPS, c % GROUPS
        cols = slice(g * EW, (g + 1) * EW)
        m = {
            "xT": xT[b],
            "cT": cT[b],
            "wq": np.ascontiguousarray(w_q[:, cols]).astype(bf16),
            "wk": np.ascontiguousarray(w_k[:, cols]).astype(bf16),
            "wv": np.ascontiguousarray(w_v[:, cols]).astype(bf16),
            "wck": np.ascontiguousarray(w_ctx_k[:, cols]).astype(bf16),
            "wcv": np.ascontiguousarray(w_ctx_v[:, cols]).astype(bf16),
            "wo": np.ascontiguousarray(w_out[g * EW:(g + 1) * EW, :])
            .astype(bf16),
            "qnw": q_norm_w.reshape(HD, 1).astype(np.float32).copy(),
            "knw": k_norm_w.reshape(HD, 1).astype(np.float32).copy(),
        }
        if with_mask:
            # mask [B,1,K,S] -> transposed [S,K] per batch (fp32).
            # The kernel folds the 1/sqrt(HD) score scale into the exp
            # activation, which would scale the mask too; pre-divide so
            # exp((scores_raw + mask/SCALE) * SCALE) = exp(scores + mask).
            m["maskT"] = np.ascontiguousarray(attn_mask[b, 0].T) * (1.0 / SCALE)
        in_maps.append(m)
    return in_maps



# revision 20
# speedup vs baseline: 1.0458x; 1.0458x over previous
"""Trainium2 Bass kernel for DFlashAttentionV5.

Reference computation (fp32, single device):
    Q/K/V/Kctx/Vctx projections -> rmsnorm(Q), rmsnorm(K_full) -> softmax
    attention over concat(ctx, self) keys/values -> output projection.

Sharding over 8 NeuronCores: batch (2-way) x head-group (4-way).
Core c handles batch b = c // 4 and heads 4*g..4*g+3 where g = c % 4.
Each core computes attention for its 4 local heads, then the transposed
full-width output-projection partial outT[2048 cols, 1024 tokens] (same
PE cost as a 512-col slice: the contraction shrinks to the 512 local E
dims), and a single ReduceScatter sums the partials across the 4-core
batch group, leaving each core its 512-row slice of outT. The collective
cost model charges ~15us fixed + out_bytes/40GBps per op, so one 1MB-out
ReduceScatter (~41us) beats four chained 1MB-out AllGathers (~165us).

Pipeline notes (cost-model driven):
  - projections: rmsnorm of column-chunk c is emitted in two deferred
    stages during chunks c+1 / c+2 so its serial ACT/DVE chain never
    stalls the PE matmul stream.
  - attention per head: phase A emits scores+exp+denominator-accumulate
    for all 24 key-chunks, phase B the 24 PV matmuls (probs are kept in
    SBUF), so PSUM only needs 3 score slots + 1 PV slot; the softmax
    denominator accumulates in two interleaved bf16 SBUF accumulators
    (fast DVE mode, no serial-chain pacing) which are partition-reduced
    by PE ones-matmuls; each head's denominator->normalize chain is
    deferred into the next head's phase A.

All matmuls run in bf16 (fp32 PSUM accumulation); softmax statistics are
fp32. x/ctx arrive pre-transposed from the host ([D, tokens]) so no
on-device transposes are needed.

Self-contained: hardcodes all shapes; only imports concourse + numpy.
"""

import math

import numpy as np
import ml_dtypes

import concourse.bass as bass
import concourse.mybir as mybir
import concourse.tile as tile
from concourse.bass_utils import run_bass_kernel_spmd

BF16 = mybir.dt.bfloat16
F32 = mybir.dt.float32
AF = mybir.ActivationFunctionType
ALU = mybir.AluOpType

# Problem dims
B, K, CTX, D, H, HD = 2, 1024, 2048, 2048, 16, 128
S = CTX + K            # 3072 keys per query
NCORES = 8
GROUPS = 4             # head groups (tensor-parallel within a batch)
NH = H // GROUPS       # 4 local heads per core
E = H * HD             # 2048
EW = NH * HD           # 512 local attention width / weight shard width
DCH = D // 128         # 16 contraction chunks
SCH = S // 128         # 24 key chunks
TCH = K // 128         # 8 query-token chunks
SCALE = 1.0 / math.sqrt(HD)
EPS = 1e-6
REPLICA_GROUPS = [[0, 1, 2, 3], [4, 5, 6, 7]]

_CACHE = {}


def _build(with_mask: bool):
    """Build the SPMD bass program (same program on all 8 cores)."""
    nc = bass.Bass(num_devices=NCORES)

    xT_d = nc.declare_dram_parameter("xT", [D, K], BF16, isOutput=False)
    cT_d = nc.declare_dram_parameter("cT", [D, CTX], BF16, isOutput=False)
    wq_d = nc.declare_dram_parameter("wq", [D, EW], BF16, isOutput=False)
    wk_d = nc.declare_dram_parameter("wk", [D, EW], BF16, isOutput=False)
    wv_d = nc.declare_dram_parameter("wv", [D, EW], BF16, isOutput=False)
    wck_d = nc.declare_dram_parameter("wck", [D, EW], BF16, isOutput=False)
    wcv_d = nc.declare_dram_parameter("wcv", [D, EW], BF16, isOutput=False)
    wo_d = nc.declare_dram_parameter("wo", [EW, E], BF16, isOutput=False)
    qnw_d = nc.declare_dram_parameter("qnw", [HD, 1], F32, isOutput=False)
    knw_d = nc.declare_dram_parameter("knw", [HD, 1], F32, isOutput=False)
    if with_mask:
        mt_d = nc.declare_dram_parameter("maskT", [S, K], F32, isOutput=False)
    # transposed output slice [out-cols 512, tokens 1024], host transposes
    out_d = nc.declare_dram_parameter("out", [EW, K], BF16, isOutput=True)

    with tile.TileContext(nc, num_cores=NCORES) as tc:
        with (
            tc.tile_pool(name="const", bufs=1) as constp,
            tc.tile_pool(name="perm", bufs=1) as perm,
            tc.tile_pool(name="stat", bufs=2) as statp,
            tc.tile_pool(name="bc", bufs=2) as bcp,
            tc.tile_pool(name="dram", bufs=1, space="DRAM") as dram,
        ):
            ones_col = constp.tile([128, 1], BF16)
            nc.any.memset(ones_col, 1.0)
            ones_row = constp.tile([1, 128], BF16)
            nc.any.memset(ones_row, 1.0)
            qnw_sb = constp.tile([HD, 1], F32)
            knw_sb = constp.tile([HD, 1], F32)

            # Resident tensors (bf16):
            #   K_sb[h]  [128=hd, 3072=s] per local head (ctx keys then self)
            #   V_sb[s]  [128=s-chunk, 512=4 heads x hd], s 0..15 ctx, 16..23 self
            #   QT_sb[h] [128=hd, 1024=q]
            K_sb = [perm.tile([128, S], BF16, tag=f"K{h}", bufs=1, name=f"K{h}")
                    for h in range(NH)]
            V_sb = [perm.tile([128, EW], BF16, tag=f"V{s}", bufs=1, name=f"V{s}")
                    for s in range(SCH)]
            QT_sb = [perm.tile([128, K], BF16, tag=f"Q{h}", bufs=1, name=f"Q{h}")
                     for h in range(NH)]
            attnT_sb = [perm.tile([128, K], BF16, tag=f"A{h}", bufs=1,
                                  name=f"A{h}") for h in range(NH)]

            # full-width transposed out-proj partial, reduce-scattered
            partial_d = dram.tile([E, K], BF16, name="partial")
            rs_out_d = dram.tile([EW, K], BF16, name="rsout")

            # ================= projection phase =================
            with (
                tc.tile_pool(name="srcT", bufs=1) as srcTp,
                tc.tile_pool(name="wstream", bufs=4) as wstream,
                tc.tile_pool(name="wwide", bufs=1) as wwide,
                tc.tile_pool(name="sqp", bufs=2) as sqp,
                tc.tile_pool(name="psA", bufs=3, space="PSUM") as psA,
                tc.tile_pool(name="ps1", bufs=2, space="PSUM") as ps1,
            ):
                # ---- rmsnorm in transposed layout, software-pipelined.
                # ps [128=hd, 1024=tokens] fp32 PSUM -> dest bf16 SBUF.
                # stage0 (immediate): square on ACT.
                # stage1 (next chunk): mean via ones-matmul, +eps, recip,
                #   rsqrt (all short ops off the PE except one tiny matmul).
                # stage2 (chunk after): broadcast matmul, copy, apply.
                def rms_stage0(ps):
                    sqt = sqp.tile([128, 1024], BF16, tag="sq")
                    nc.scalar.square(sqt[:], ps[:])
                    return sqt

                def rms_stage1(sqt):
                    rss = []
                    for j in range(2):
                        js = slice(j * 512, (j + 1) * 512)
                        ps_s = ps1.tile([128, 512], F32, tag="ps1")
                        nc.tensor.matmul(ps_s[0:1, :], ones_col[:], sqt[:, js],
                                         start=True, stop=True)
                        mean = statp.tile([1, 512], F32, tag="mean")
                        nc.vector.tensor_scalar(mean[:], ps_s[0:1, :],
                                                1.0 / HD, EPS,
                                                ALU.mult, ALU.add)
                        rec = statp.tile([1, 512], F32, tag="rec")
                        nc.vector.reciprocal(rec[:], mean[:])
                        rs = statp.tile([1, 512], BF16, tag="rs")
                        nc.scalar.sqrt(rs[:], rec[:])  # rsqrt = sqrt(1/x)
                        rss.append(rs)
                    return rss

                def rms_stage2(ps, rss, dest_ap, nw_sb):
                    for j in range(2):
                        js = slice(j * 512, (j + 1) * 512)
                        ps_b = ps1.tile([128, 512], F32, tag="ps1")
                        nc.tensor.matmul(ps_b[:], ones_row[:], rss[j][:],
                                         start=True, stop=True)
                        bc = bcp.tile([128, 512], F32, tag="bc")
                        nc.scalar.copy(bc[:], ps_b[:])
                        nc.vector.scalar_tensor_tensor(
                            dest_ap[:, js], ps[:, js], nw_sb[:], bc[:],
                            ALU.mult, ALU.mult)

                # deferred-emission pipeline: stage1(c) emits during chunk
                # c+1, stage2(c) during chunk c+2, so the serial rms chain
                # always has a full chunk of matmuls to hide behind.
                pend1 = []  # stage1 closures; popped -> emit, yields stage2
                pend2 = []  # stage2 closures awaiting emission

                def boundary():
                    if pend2:
                        pend2.pop(0)()
                    if pend1:
                        pend2.append(pend1.pop(0)())

                def flush():
                    while pend1 or pend2:
                        boundary()

                # d-chunk accessor over grouped source tiles [128, 4*1024]
                def src_at(grp, d):
                    return grp[d // 4], (d % 4) * 1024

                def load_set(grp, dram_ap, split_first=False):
                    # dram_ap: [D, 1024] (d-major); one DMA per 4 d-chunks.
                    # split_first peels d-chunk 0 into its own small DMA so
                    # the first dependent matmul can start sooner.
                    if split_first:
                        nc.sync.dma_start(
                            grp[0][:, 0:1024],
                            dram_ap[0:128, :])
                        nc.sync.dma_start(
                            grp[0][:, 1024:4096].rearrange("p (a t) -> p a t",
                                                           t=1024),
                            dram_ap[128:512, :]
                            .rearrange("(a p) t -> p a t", p=128))
                    else:
                        nc.sync.dma_start(
                            grp[0][:].rearrange("p (a t) -> p a t", t=1024),
                            dram_ap[0:512, :]
                            .rearrange("(a p) t -> p a t", p=128))
                    for i in range(1, 4):
                        nc.sync.dma_start(
                            grp[i][:].rearrange("p (a t) -> p a t", t=1024),
                            dram_ap[i * 512:(i + 1) * 512, :]
                            .rearrange("(a p) t -> p a t", p=128))

                # Q^T / K^T projections (weight-stationary):
                # psum[c] [128=col-chunk(head), 1024 tokens] += w[d,c].T @ srcT[d]
                def load_wchunk(w_d, c, name=None):
                    wch = wstream.tile([128, D], BF16, tag="w", name=name)
                    nc.sync.dma_start(
                        wch[:].rearrange("p (a q) -> p a q", q=128),
                        w_d[:, c * 128:(c + 1) * 128]
                        .rearrange("(a p) q -> p a q", p=128))
                    return wch

                def qk_proj(w_d, srcT, dest_of_chunk, nw_sb, pre=None):
                    for c in range(EW // 128):
                        wch = pre if (c == 0 and pre is not None) \
                            else load_wchunk(w_d, c)
                        ps = psA.tile([128, 1024], F32, tag="psA",
                                      name=f"psqk{c}")
                        for d in range(DCH):
                            st, off = src_at(srcT, d)
                            for j in range(2):
                                nc.tensor.matmul(
                                    ps[:, j * 512:(j + 1) * 512],
                                    wch[:, d * 128:(d + 1) * 128],
                                    st[:, off + j * 512:off + j * 512 + 512],
                                    start=(d == 0), stop=(d == DCH - 1))
                        boundary()
                        sqt = rms_stage0(ps)
                        dest, off = dest_of_chunk(c)

                        def s1(sqt=sqt, ps=ps, dest=dest, off=off, nw=nw_sb):
                            rss = rms_stage1(sqt)
                            return lambda: rms_stage2(
                                ps, rss, dest[:, off:off + 1024], nw)

                        pend1.append(s1)

                # V projections (activation-stationary):
                # V_sb[s] [128=tokens, 512=cols] += srcT[d][:,t-chunk].T @ wv[d]
                def load_wide(w_d, tag):
                    grp = []
                    for i in range(4):
                        wt = wwide.tile([128, 4 * EW], BF16, tag=f"wv{i}",
                                        bufs=1, name=f"wv{i}_{tag}")
                        nc.sync.dma_start(
                            wt[:].rearrange("p (a q) -> p a q", q=EW),
                            w_d[i * 512:(i + 1) * 512, :]
                            .rearrange("(a p) q -> p a q", p=128))
                        grp.append(wt)
                    return grp

                def v_proj(wv_grp, srcT, s_base):
                    for t in range(TCH):
                        ps = ps1.tile([128, 512], F32, tag="ps1")
                        for d in range(DCH):
                            st, off = src_at(srcT, d)
                            wvt = wv_grp[d // 4]
                            wo_off = (d % 4) * EW
                            nc.tensor.matmul(
                                ps[:], st[:, off + t * 128:off + (t + 1) * 128],
                                wvt[:, wo_off:wo_off + EW],
                                start=(d == 0), stop=(d == DCH - 1))
                        nc.vector.tensor_copy(V_sb[s_base + t][:], ps[:])
                        boundary()

                # slot sets: A = xT then ctx-half1 (recycled), B = ctx-half0
                setA = [srcTp.tile([128, 4096], BF16, tag=f"sa{i}", bufs=1,
                                   name=f"xT{i}") for i in range(4)]
                setB = [srcTp.tile([128, 4096], BF16, tag=f"sb{i}", bufs=1,
                                   name=f"cTa{i}") for i in range(4)]
                pre_wq = load_wchunk(wq_d, 0, name="prewq")
                load_set(setA, xT_d, split_first=True)
                nc.sync.dma_start(qnw_sb[:], qnw_d[:])
                nc.sync.dma_start(knw_sb[:], knw_d[:])

                # self tokens (block A): Q, K_self, V_self
                qk_proj(wq_d, setA, lambda c: (QT_sb[c], 0), qnw_sb, pre=pre_wq)
                load_set(setB, cT_d[:, 0:1024])
                wv_sb = load_wide(wv_d, "s")
                qk_proj(wk_d, setA, lambda c: (K_sb[c], CTX), knw_sb)
                v_proj(wv_sb, setA, CTX // 128)

                # ctx half 0 (block B): K_ctx[:, 0:1024], V_ctx s-chunks 0..7
                wcv_sb = load_wide(wcv_d, "c0")
                qk_proj(wck_d, setB, lambda c: (K_sb[c], 0), knw_sb)
                v_proj(wcv_sb, setB, 0)

                # ctx half 1 reuses set A slots
                setC = [srcTp.tile([128, 4096], BF16, tag=f"sa{i}", bufs=1,
                                   name=f"cTb{i}") for i in range(4)]
                load_set(setC, cT_d[:, 1024:2048])
                wcv2_sb = load_wide(wcv_d, "c1")
                qk_proj(wck_d, setC, lambda c: (K_sb[c], 1024), knw_sb)
                v_proj(wcv2_sb, setC, TCH)
                flush()

            # ================= attention + output phase =================
            with (
                tc.tile_pool(name="probsT", bufs=24) as probsp,
                tc.tile_pool(name="acc", bufs=2) as accp,
                tc.tile_pool(name="wop", bufs=1) as wop,
                tc.tile_pool(name="ostg", bufs=3) as ostgp,
                tc.tile_pool(name="mrow", bufs=4) as mrowp,
                tc.tile_pool(name="pst", bufs=3, space="PSUM") as pstp,
                tc.tile_pool(name="pvp", bufs=1, space="PSUM") as pvp,
            ):
                # w_out rows for local head h (global rows (g*NH+h)*128..),
                # all 2048 columns: [128=hd, 2048=out-cols]
                wo_sb = [wop.tile([128, E], BF16, tag=f"wo{h}", bufs=1,
                                  name=f"wo{h}") for h in range(NH)]
                for h in range(NH):
                    nc.sync.dma_start(wo_sb[h][:],
                                      wo_d[h * 128:(h + 1) * 128, :])

                # softmax denominator + normalize for one finished head:
                # column sums of the two bf16 accumulators via PE ones-
                # matmuls (PSUM-accumulated), reciprocal, broadcast across
                # partitions via K=1 matmul, then attnT = pv * bcast(1/den).
                def denom(h, pv, aE, aO):
                    ps_d = pstp.tile([1, K], F32, tag="st", name=f"psd{h}")
                    for j in range(2):
                        js = slice(j * 512, (j + 1) * 512)
                        nc.tensor.matmul(ps_d[0:1, js], ones_col[:],
                                         aE[:, js], start=True, stop=False)
                        nc.tensor.matmul(ps_d[0:1, js], ones_col[:],
                                         aO[:, js], start=False, stop=True)
                    rec = statp.tile([1, K], F32, tag="recA")
                    nc.vector.reciprocal(rec[:], ps_d[0:1, :])
                    rb = statp.tile([1, K], BF16, tag="rbA")
                    nc.vector.tensor_copy(rb[:], rec[:])
                    ps_b = pstp.tile([128, K], F32, tag="st", name=f"psb{h}")
                    for j in range(2):
                        js = slice(j * 512, (j + 1) * 512)
                        nc.tensor.matmul(ps_b[:, js], ones_row[:],
                                         rb[0:1, js], start=True, stop=True)
                    bc = bcp.tile([128, K], F32, tag="bcA")
                    nc.scalar.copy(bc[:], ps_b[:])
                    for j in range(2):
                        js = slice(j * 512, (j + 1) * 512)
                        nc.vector.tensor_tensor(attnT_sb[h][:, js],
                                                pv[:, js], bc[:, js],
                                                ALU.mult)

                # attention for one local head, transposed scores:
                # scoresT[s-chunk] [128=s, 1024=q] = K_chunk @ Q^T  (no max
                # subtraction: scores ~ N(0,1) after rmsnorm + 1/sqrt(HD)).
                # Phase A: scores+exp+acc per key chunk; probs stay in SBUF.
                # Phase B: the 24 PV accumulation matmuls.
                def attention(h, post_cb=None):
                    aE = accp.tile([128, K], BF16, tag="aE", name=f"aE{h}")
                    aO = accp.tile([128, K], BF16, tag="aO", name=f"aO{h}")
                    pv = pvp.tile([128, K], F32, tag="pv", name=f"pv{h}")
                    pts = []
                    for s in range(SCH):
                        st = pstp.tile([128, K], F32, tag="st",
                                       name=f"sT{h}_{s}")
                        for j in range(2):
                            js = slice(j * 512, (j + 1) * 512)
                            nc.tensor.matmul(
                                st[:, js], K_sb[h][:, s * 128:(s + 1) * 128],
                                QT_sb[h][:, js], start=True, stop=True)
                        if with_mask:
                            mrow = mrowp.tile([128, K], F32, tag="mrow")
                            nc.sync.dma_start(
                                mrow[:], mt_d[s * 128:(s + 1) * 128, :])
                            nc.vector.tensor_tensor(st[:], st[:], mrow[:],
                                                    ALU.add)
                        pT = probsp.tile([128, K], BF16, tag="pT",
                                         name=f"pT{h}_{s}")
                        nc.scalar.activation(pT[:], st[:], AF.Exp, scale=SCALE)
                        acc = aE if s % 2 == 0 else aO
                        if s < 2:
                            nc.vector.tensor_copy(acc[:], pT[:])
                        else:
                            nc.vector.tensor_tensor(acc[:], acc[:], pT[:],
                                                    ALU.add)
                        pts.append(pT)
                        if s == 2 and post_cb is not None:
                            post_cb()
                    for s in range(SCH):
                        for j in range(2):
                            js = slice(j * 512, (j + 1) * 512)
                            nc.tensor.matmul(
                                pv[:, js], V_sb[s][:, h * 128:(h + 1) * 128],
                                pts[s][:, js],
                                start=(s == 0), stop=(s == SCH - 1))
                    return (h, pv, aE, aO)

                state = None
                for h in range(NH):
                    cb = None
                    if state is not None:
                        cb = (lambda st=state: denom(*st))
                    state = attention(h, post_cb=cb)
                # last head's denominator: phase A finished a full phase-B
                # ago, so the chain runs entirely under the out-proj matmuls
                denom(*state)

                # transposed out-proj partial: for each 128-col chunk c of E,
                # outT[c][128 cols, 1024 tok] = sum_h wo_sb[h][:,c].T@attnT[h]
                # (PSUM-accumulated over the 4 local heads; h3 last so the
                # just-emitted denom(h3) chain hides under h0-h2 matmuls),
                # staged to bf16 (alternating DVE/ACT) and DMAd to DRAM.
                for c in range(E // 128):
                    ps = pstp.tile([128, K], F32, tag="st", name=f"ot{c}")
                    for h in range(NH):
                        for j in range(2):
                            js = slice(j * 512, (j + 1) * 512)
                            nc.tensor.matmul(
                                ps[:, js], wo_sb[h][:, c * 128:(c + 1) * 128],
                                attnT_sb[h][:, js],
                                start=(h == 0), stop=(h == NH - 1))
                    stg = ostgp.tile([128, K], BF16, tag="ostg")
                    if c % 2 == 0:
                        nc.vector.tensor_copy(stg[:], ps[:])
                    else:
                        nc.scalar.copy(stg[:], ps[:])
                    nc.sync.dma_start(partial_d[c * 128:(c + 1) * 128, :],
                                      stg[:])

                nc.gpsimd.collective_compute(
                    "ReduceScatter", ALU.add,
                    replica_groups=REPLICA_GROUPS,
                    ins=[partial_d[:].opt()],
                    outs=[rs_out_d[:].opt()],
                )
                nc.sync.dma_start(out_d[:], rs_out_d[:])

    return nc


def _split_multiwaits(nc):
    """walrus codegen in this container rejects instructions with more than
    one semaphore wait; split the excess onto preceding NoOps on the same
    engine."""
    for f in nc.m.functions:
        for blk in f.blocks:
            idx = 0
            while idx < len(blk.instructions):
                inst = blk.instructions[idx]
                si = inst.sync_info
                maxw = 1
                if si is None or len(si.on_wait) <= maxw:
                    idx += 1
                    continue
                waits = list(si.on_wait)
                ncarry = (len(waits) - 1) // maxw  # leave <=maxw on inst
                for k in range(ncarry):
                    chunk = waits[k * maxw:(k + 1) * maxw]
                    nop = mybir.InstNoOp(
                        name=nc.get_next_instruction_name(),
                        ins=[], outs=[],
                        bass_nofuse=True,
                        sync_info=mybir.SyncInfo(on_wait=chunk, on_update=[]),
                    )
                    nop.engine = inst.engine
                    nc.register_instruction(nop)
                    blk.instructions.insert(idx, nop)
                    idx += 1
                si.on_wait = waits[ncarry * maxw:]
                idx += 1


def _get_program(with_mask: bool):
    key = ("prog", with_mask)
    if key not in _CACHE:
        nc = _build(with_mask)
        _split_multiwaits(nc)
        _CACHE[key] = nc
    return _CACHE[key]


def kernel(x, context, attn_mask, w_q, w_k, w_v, w_ctx_k, w_ctx_v, w_out,
           q_norm_w, k_norm_w):
    x = np.asarray(x, np.float32)
    context = np.asarray(context, np.float32)
    attn_mask = np.asarray(attn_mask, np.float32)
    w_q = np.asarray(w_q, np.float32)
    w_k = np.asarray(w_k, np.float32)
    w_v = np.asarray(w_v, np.float32)
    w_ctx_k = np.asarray(w_ctx_k, np.float32)
    w_ctx_v = np.asarray(w_ctx_v, np.float32)
    w_out = np.asarray(w_out, np.float32)
    q_norm_w = np.asarray(q_norm_w, np.float32)
    k_norm_w = np.asarray(k_norm_w, np.float32)

    with_mask = bool(np.any(attn_mask))
    nc = _get_program(with_mask)
    in_maps = _prepare_in_maps(x, context, attn_mask, w_q, w_k, w_v, w_ctx_k,
                               w_ctx_v, w_out, q_norm_w, k_norm_w, with_mask)

    res = run_bass_kernel_spmd(nc, in_maps, list(range(NCORES))).results
    return _assemble(res)


def _assemble(res):
    out = np.empty((B, K, D), np.float32)
    for c in range(NCORES):
        b, g = c // GROUPS, c % GROUPS
        out[b, :, g * EW:(g + 1) * EW] = res[c]["out"].astype(np.float32).T
    return out


def _prepare_in_maps(x, context, attn_mask, w_q, w_k, w_v, w_ctx_k, w_ctx_v,
                     w_out, q_norm_w, k_norm_w, with_mask):
    bf16 = ml_dtypes.bfloat16
    xT = [np.ascontiguousarray(x[b].T).astype(bf16) for b in range(B)]
    cT = [np.ascontiguousarray(context[b].T).astype(bf16) for b in range(B)]
    in_maps = []
    for c in range(NCORES):
        b, g = c // GROUPS, c % GROUPS
        cols = slice(g * EW, (g + 1) * EW)
        m = {
            "xT": xT[b],
            "cT": cT[b],
            "wq": np.ascontiguousarray(w_q[:, cols]).astype(bf16),
            "wk": np.ascontiguousarray(w_k[:, cols]).astype(bf16),
            "wv": np.ascontiguousarray(w_v[:, cols]).astype(bf16),
            "wck": np.ascontiguousarray(w_ctx_k[:, cols]).astype(bf16),
            "wcv": np.ascontiguousarray(w_ctx_v[:, cols]).astype(bf16),
            "wo": np.ascontiguousarray(w_out[g * EW:(g + 1) * EW, :])
            .astype(bf16),
            "qnw": q_norm_w.reshape(HD, 1).astype(np.float32).copy(),
            "knw": k_norm_w.reshape(HD, 1).astype(np.float32).copy(),
        }
        if with_mask:
            # mask [B,1,K,S] -> transposed [S,K] per batch (fp32).
            # The kernel folds the 1/sqrt(HD) score scale into the exp
            # activation, which would scale the mask too; pre-divide so
            # exp((scores_raw + mask/SCALE) * SCALE) = exp(scores + mask).
            m["maskT"] = np.ascontiguousarray(attn_mask[b, 0].T) * (1.0 / SCALE)
        in_maps.append(m)
    return in_maps


# revision 25
# speedup vs baseline: 1.1019x; 1.0537x over previous
"""Trainium2 Bass kernel for DFlashAttentionV5.

Reference computation (fp32, single device):
    Q/K/V/Kctx/Vctx projections -> rmsnorm(Q), rmsnorm(K_full) -> softmax
    attention over concat(ctx, self) keys/values -> output projection.

Sharding over 8 NeuronCores: batch (2-way) x head-group (4-way).
Core c handles batch b = c // 4 and heads 4*g..4*g+3 where g = c % 4.
Each core computes attention for its 4 local heads, then the transposed
full-width output-projection partial outT[2048 cols, 1024 tokens] (same
PE cost as a 512-col slice: the contraction shrinks to the 512 local E
dims), and a single ReduceScatter sums the partials across the 4-core
batch group, leaving each core its 512-row slice of outT. The collective
cost model charges ~15us fixed + out_bytes/40GBps per op, so one 1MB-out
ReduceScatter (~41us) beats four chained 1MB-out AllGathers (~165us).

Pipeline notes (cost-model driven):
  - projections: rmsnorm of column-chunk c is emitted in two deferred
    stages during chunks c+1 / c+2 so its serial ACT/DVE chain never
    stalls the PE matmul stream.
  - attention per head: phase A emits scores+exp+denominator-accumulate
    for all 24 key-chunks, phase B the 24 PV matmuls (probs are kept in
    SBUF), so PSUM only needs 3 score slots + 1 PV slot; the softmax
    denominator accumulates in two interleaved bf16 SBUF accumulators
    (fast DVE mode, no serial-chain pacing) which are partition-reduced
    by PE ones-matmuls; each head's denominator->normalize chain is
    deferred into the next head's phase A.

All matmuls run in bf16 (fp32 PSUM accumulation); softmax statistics are
fp32. x/ctx arrive pre-transposed from the host ([D, tokens]) so no
on-device transposes are needed.

Self-contained: hardcodes all shapes; only imports concourse + numpy.
"""

import math

import numpy as np
import ml_dtypes

import concourse.bass as bass
import concourse.mybir as mybir
import concourse.tile as tile
from concourse.bass_utils import run_bass_kernel_spmd

BF16 = mybir.dt.bfloat16
F32 = mybir.dt.float32
AF = mybir.ActivationFunctionType
ALU = mybir.AluOpType

# Problem dims
B, K, CTX, D, H, HD = 2, 1024, 2048, 2048, 16, 128
S = CTX + K            # 3072 keys per query
NCORES = 8
GROUPS = 4             # head groups (tensor-parallel within a batch)
NH = H // GROUPS       # 4 local heads per core
E = H * HD             # 2048
EW = NH * HD           # 512 local attention width / weight shard width
DCH = D // 128         # 16 contraction chunks
SCH = S // 128         # 24 key chunks
TCH = K // 128         # 8 query-token chunks
SCALE = 1.0 / math.sqrt(HD)
EPS = 1e-6
REPLICA_GROUPS = [[0, 1, 2, 3], [4, 5, 6, 7]]

_CACHE = {}


def _build(with_mask: bool):
    """Build the SPMD bass program (same program on all 8 cores)."""
    nc = bass.Bass(num_devices=NCORES)

    xT_d = nc.declare_dram_parameter("xT", [D, K], BF16, isOutput=False)
    cT_d = nc.declare_dram_parameter("cT", [D, CTX], BF16, isOutput=False)
    wq_d = nc.declare_dram_parameter("wq", [D, EW], BF16, isOutput=False)
    wk_d = nc.declare_dram_parameter("wk", [D, EW], BF16, isOutput=False)
    wv_d = nc.declare_dram_parameter("wv", [D, EW], BF16, isOutput=False)
    wck_d = nc.declare_dram_parameter("wck", [D, EW], BF16, isOutput=False)
    wcv_d = nc.declare_dram_parameter("wcv", [D, EW], BF16, isOutput=False)
    wo_d = nc.declare_dram_parameter("wo", [EW, E], BF16, isOutput=False)
    qnw_d = nc.declare_dram_parameter("qnw", [HD, 1], F32, isOutput=False)
    knw_d = nc.declare_dram_parameter("knw", [HD, 1], F32, isOutput=False)
    if with_mask:
        mt_d = nc.declare_dram_parameter("maskT", [S, K], F32, isOutput=False)
    # transposed output slice [out-cols 512, tokens 1024], host transposes
    out_d = nc.declare_dram_parameter("out", [EW, K], BF16, isOutput=True)

    with tile.TileContext(nc, num_cores=NCORES) as tc:
        with (
            tc.tile_pool(name="const", bufs=1) as constp,
            tc.tile_pool(name="perm", bufs=1) as perm,
            tc.tile_pool(name="stat", bufs=2) as statp,
            tc.tile_pool(name="bc", bufs=2) as bcp,
            tc.tile_pool(name="dram", bufs=1, space="DRAM") as dram,
        ):
            ones_col = constp.tile([128, 1], BF16)
            nc.any.memset(ones_col, 1.0)
            ones_row = constp.tile([1, 128], BF16)
            nc.any.memset(ones_row, 1.0)
            qnw_sb = constp.tile([HD, 1], F32)
            knw_sb = constp.tile([HD, 1], F32)

            # Resident tensors (bf16):
            #   K_sb[h]  [128=hd, 3072=s] per local head (ctx keys then self)
            #   V_sb[s]  [128=s-chunk, 512=4 heads x hd], s 0..15 ctx, 16..23 self
            #   QT_sb[h] [128=hd, 1024=q]
            K_sb = [perm.tile([128, S], BF16, tag=f"K{h}", bufs=1, name=f"K{h}")
                    for h in range(NH)]
            V_sb = [perm.tile([128, EW], BF16, tag=f"V{s}", bufs=1, name=f"V{s}")
                    for s in range(SCH)]
            QT_sb = [perm.tile([128, K], BF16, tag=f"Q{h}", bufs=1, name=f"Q{h}")
                     for h in range(NH)]
            attnT_sb = [perm.tile([128, K], BF16, tag=f"A{h}", bufs=1,
                                  name=f"A{h}") for h in range(NH)]

            # full-width transposed out-proj partial, reduce-scattered
            partial_d = dram.tile([E, K], BF16, name="partial")
            rs_out_d = dram.tile([EW, K], BF16, name="rsout")

            # ================= projection phase =================
            with (
                tc.tile_pool(name="srcT", bufs=1) as srcTp,
                tc.tile_pool(name="wstream", bufs=4) as wstream,
                tc.tile_pool(name="wwide", bufs=1) as wwide,
                tc.tile_pool(name="sqp", bufs=2) as sqp,
                tc.tile_pool(name="psA", bufs=3, space="PSUM") as psA,
                tc.tile_pool(name="ps1", bufs=2, space="PSUM") as ps1,
            ):
                # ---- rmsnorm in transposed layout, software-pipelined.
                # ps [128=hd, 1024=tokens] fp32 PSUM -> dest bf16 SBUF.
                # stage0 (immediate): square on ACT.
                # stage1 (next chunk): mean via ones-matmul, +eps, recip,
                #   rsqrt (all short ops off the PE except one tiny matmul).
                # stage2 (chunk after): broadcast matmul, copy, apply.
                def rms_stage0(ps):
                    # square for the mean, plus a bf16 snapshot of the
                    # projection values so stage2 reads SBUF and the PSUM
                    # tile retires immediately (keeps the psA ring free)
                    sqt = sqp.tile([128, 1024], BF16, tag="sq")
                    nc.scalar.square(sqt[:], ps[:])
                    psb = sqp.tile([128, 1024], BF16, tag="psb", bufs=3)
                    nc.vector.tensor_copy(psb[:], ps[:])
                    return sqt, psb

                def rms_stage1(sqt):
                    rss = []
                    for j in range(2):
                        js = slice(j * 512, (j + 1) * 512)
                        ps_s = ps1.tile([128, 512], F32, tag="ps1")
                        nc.tensor.matmul(ps_s[0:1, :], ones_col[:], sqt[:, js],
                                         start=True, stop=True)
                        mean = statp.tile([1, 512], F32, tag="mean")
                        nc.vector.tensor_scalar(mean[:], ps_s[0:1, :],
                                                1.0 / HD, EPS,
                                                ALU.mult, ALU.add)
                        rec = statp.tile([1, 512], F32, tag="rec")
                        nc.vector.reciprocal(rec[:], mean[:])
                        rs = statp.tile([1, 512], BF16, tag="rs")
                        nc.scalar.sqrt(rs[:], rec[:])  # rsqrt = sqrt(1/x)
                        rss.append(rs)
                    return rss

                def rms_stage2(psb, rss, dest_ap, nw_sb):
                    for j in range(2):
                        js = slice(j * 512, (j + 1) * 512)
                        ps_b = ps1.tile([128, 512], F32, tag="ps1")
                        nc.tensor.matmul(ps_b[:], ones_row[:], rss[j][:],
                                         start=True, stop=True)
                        bc = bcp.tile([128, 512], F32, tag="bc")
                        nc.scalar.copy(bc[:], ps_b[:])
                        nc.vector.scalar_tensor_tensor(
                            dest_ap[:, js], psb[:, js], nw_sb[:], bc[:],
                            ALU.mult, ALU.mult)

                # deferred-emission pipeline: stage1(c) emits during chunk
                # c+1, stage2(c) during chunk c+2, so the serial rms chain
                # always has a full chunk of matmuls to hide behind.
                pend1 = []  # stage1 closures; popped -> emit, yields stage2
                pend2 = []  # stage2 closures awaiting emission

                def boundary():
                    # stage1(c-1) first: its tiny PE ops depend only on the
                    # long-finished square; stage2(c-2)'s broadcast matmul
                    # then finds its rsqrt row already computed.
                    new2 = pend1.pop(0)() if pend1 else None
                    if pend2:
                        pend2.pop(0)()
                    if new2 is not None:
                        pend2.append(new2)

                def flush():
                    while pend1 or pend2:
                        boundary()

                # d-chunk accessor over grouped source tiles [128, 4*1024]
                def src_at(grp, d):
                    return grp[d // 4], (d % 4) * 1024

                def load_set(grp, dram_ap, split_first=False):
                    # dram_ap: [D, 1024] (d-major); one DMA per 4 d-chunks.
                    # split_first peels d-chunk 0 into its own small DMA so
                    # the first dependent matmul can start sooner.
                    if split_first:
                        nc.sync.dma_start(
                            grp[0][:, 0:1024],
                            dram_ap[0:128, :])
                        nc.sync.dma_start(
                            grp[0][:, 1024:4096].rearrange("p (a t) -> p a t",
                                                           t=1024),
                            dram_ap[128:512, :]
                            .rearrange("(a p) t -> p a t", p=128))
                    else:
                        nc.sync.dma_start(
                            grp[0][:].rearrange("p (a t) -> p a t", t=1024),
                            dram_ap[0:512, :]
                            .rearrange("(a p) t -> p a t", p=128))
                    for i in range(1, 4):
                        nc.sync.dma_start(
                            grp[i][:].rearrange("p (a t) -> p a t", t=1024),
                            dram_ap[i * 512:(i + 1) * 512, :]
                            .rearrange("(a p) t -> p a t", p=128))

                # Q^T / K^T projections (weight-stationary):
                # psum[c] [128=col-chunk(head), 1024 tokens] += w[d,c].T @ srcT[d]
                def load_wchunk(w_d, c, name=None):
                    wch = wstream.tile([128, D], BF16, tag="w", name=name)
                    nc.sync.dma_start(
                        wch[:].rearrange("p (a q) -> p a q", q=128),
                        w_d[:, c * 128:(c + 1) * 128]
                        .rearrange("(a p) q -> p a q", p=128))
                    return wch

                def qk_proj(w_d, srcT, dest_of_chunk, nw_sb, pre=None):
                    for c in range(EW // 128):
                        wch = pre if (c == 0 and pre is not None) \
                            else load_wchunk(w_d, c)
                        ps = psA.tile([128, 1024], F32, tag="psA",
                                      name=f"psqk{c}")
                        for d in range(DCH):
                            st, off = src_at(srcT, d)
                            for j in range(2):
                                nc.tensor.matmul(
                                    ps[:, j * 512:(j + 1) * 512],
                                    wch[:, d * 128:(d + 1) * 128],
                                    st[:, off + j * 512:off + j * 512 + 512],
                                    start=(d == 0), stop=(d == DCH - 1))
                        boundary()
                        sqt, psb = rms_stage0(ps)
                        dest, off = dest_of_chunk(c)

                        def s1(sqt=sqt, psb=psb, dest=dest, off=off,
                               nw=nw_sb):
                            rss = rms_stage1(sqt)
                            return lambda: rms_stage2(
                                psb, rss, dest[:, off:off + 1024], nw)

                        pend1.append(s1)

                # V projections (activation-stationary):
                # V_sb[s] [128=tokens, 512=cols] += srcT[d][:,t-chunk].T @ wv[d]
                def load_wide(w_d, tag):
                    grp = []
                    for i in range(4):
                        wt = wwide.tile([128, 4 * EW], BF16, tag=f"wv{i}",
                                        bufs=1, name=f"wv{i}_{tag}")
                        nc.sync.dma_start(
                            wt[:].rearrange("p (a q) -> p a q", q=EW),
                            w_d[i * 512:(i + 1) * 512, :]
                            .rearrange("(a p) q -> p a q", p=128))
                        grp.append(wt)
                    return grp

                def v_proj(wv_grp, srcT, s_base):
                    for t in range(TCH):
                        ps = ps1.tile([128, 512], F32, tag="ps1")
                        for d in range(DCH):
                            st, off = src_at(srcT, d)
                            wvt = wv_grp[d // 4]
                            wo_off = (d % 4) * EW
                            nc.tensor.matmul(
                                ps[:], st[:, off + t * 128:off + (t + 1) * 128],
                                wvt[:, wo_off:wo_off + EW],
                                start=(d == 0), stop=(d == DCH - 1))
                        nc.vector.tensor_copy(V_sb[s_base + t][:], ps[:])
                        boundary()

                # slot sets: A = xT then ctx-half1 (recycled), B = ctx-half0
                setA = [srcTp.tile([128, 4096], BF16, tag=f"sa{i}", bufs=1,
                                   name=f"xT{i}") for i in range(4)]
                setB = [srcTp.tile([128, 4096], BF16, tag=f"sb{i}", bufs=1,
                                   name=f"cTa{i}") for i in range(4)]
                # first weight chunk: peel d-chunks 0-1 into a small DMA so
                # the very first matmul only waits ~0.5us of DMA
                pre_wq = wstream.tile([128, D], BF16, tag="w", name="prewq")
                nc.sync.dma_start(
                    pre_wq[:, 0:256].rearrange("p (a q) -> p a q", q=128),
                    wq_d[0:256, 0:128].rearrange("(a p) q -> p a q", p=128))
                nc.sync.dma_start(
                    pre_wq[:, 256:D].rearrange("p (a q) -> p a q", q=128),
                    wq_d[256:D, 0:128].rearrange("(a p) q -> p a q", p=128))
                load_set(setA, xT_d, split_first=True)
                nc.sync.dma_start(qnw_sb[:], qnw_d[:])
                nc.sync.dma_start(knw_sb[:], knw_d[:])

                # self tokens (block A): Q, K_self, V_self
                qk_proj(wq_d, setA, lambda c: (QT_sb[c], 0), qnw_sb, pre=pre_wq)
                load_set(setB, cT_d[:, 0:1024])
                wv_sb = load_wide(wv_d, "s")
                qk_proj(wk_d, setA, lambda c: (K_sb[c], CTX), knw_sb)
                v_proj(wv_sb, setA, CTX // 128)

                # ctx half 0 (block B): K_ctx[:, 0:1024], V_ctx s-chunks 0..7
                wcv_sb = load_wide(wcv_d, "c0")
                qk_proj(wck_d, setB, lambda c: (K_sb[c], 0), knw_sb)
                v_proj(wcv_sb, setB, 0)

                # ctx half 1 reuses set A slots
                setC = [srcTp.tile([128, 4096], BF16, tag=f"sa{i}", bufs=1,
                                   name=f"cTb{i}") for i in range(4)]
                load_set(setC, cT_d[:, 1024:2048])
                wcv2_sb = load_wide(wcv_d, "c1")
                qk_proj(wck_d, setC, lambda c: (K_sb[c], 1024), knw_sb)
                v_proj(wcv2_sb, setC, TCH)
                flush()

            # ================= attention + output phase =================
            with (
                tc.tile_pool(name="probsT", bufs=24) as probsp,
                tc.tile_pool(name="acc", bufs=2) as accp,
                tc.tile_pool(name="wop", bufs=1) as wop,
                tc.tile_pool(name="ostg", bufs=3) as ostgp,
                tc.tile_pool(name="mrow", bufs=4) as mrowp,
                tc.tile_pool(name="pst", bufs=3, space="PSUM") as pstp,
                tc.tile_pool(name="pvp", bufs=1, space="PSUM") as pvp,
            ):
                # w_out rows for local head h (global rows (g*NH+h)*128..),
                # all 2048 columns: [128=hd, 2048=out-cols]
                wo_sb = [wop.tile([128, E], BF16, tag=f"wo{h}", bufs=1,
                                  name=f"wo{h}") for h in range(NH)]
                for h in range(NH):
                    nc.sync.dma_start(wo_sb[h][:],
                                      wo_d[h * 128:(h + 1) * 128, :])

                # softmax denominator + normalize for one finished head:
                # column sums of the two bf16 accumulators via PE ones-
                # matmuls (PSUM-accumulated), reciprocal, broadcast across
                # partitions via K=1 matmul, then attnT = pv * bcast(1/den).
                def denom(h, pv, aE, aO):
                    ps_d = pstp.tile([1, K], F32, tag="st", name=f"psd{h}")
                    for j in range(2):
                        js = slice(j * 512, (j + 1) * 512)
                        nc.tensor.matmul(ps_d[0:1, js], ones_col[:],
                                         aE[:, js], start=True, stop=False)
                        nc.tensor.matmul(ps_d[0:1, js], ones_col[:],
                                         aO[:, js], start=False, stop=True)
                    rec = statp.tile([1, K], F32, tag="recA")
                    nc.vector.reciprocal(rec[:], ps_d[0:1, :])
                    rb = statp.tile([1, K], BF16, tag="rbA")
                    nc.vector.tensor_copy(rb[:], rec[:])
                    ps_b = pstp.tile([128, K], F32, tag="st", name=f"psb{h}")
                    for j in range(2):
                        js = slice(j * 512, (j + 1) * 512)
                        nc.tensor.matmul(ps_b[:, js], ones_row[:],
                                         rb[0:1, js], start=True, stop=True)
                    bc = bcp.tile([128, K], F32, tag="bcA")
                    nc.scalar.copy(bc[:], ps_b[:])
                    for j in range(2):
                        js = slice(j * 512, (j + 1) * 512)
                        nc.vector.tensor_tensor(attnT_sb[h][:, js],
                                                pv[:, js], bc[:, js],
                                                ALU.mult)

                # attention for one local head, transposed scores:
                # scoresT[s-chunk] [128=s, 1024=q] = K_chunk @ Q^T  (no max
                # subtraction: scores ~ N(0,1) after rmsnorm + 1/sqrt(HD)).
                # Phase A: scores+exp+acc per key chunk; probs stay in SBUF.
                # Phase B: the 24 PV accumulation matmuls.
                def attention(h, post_cb=None):
                    aE = accp.tile([128, K], BF16, tag="aE", name=f"aE{h}")
                    aO = accp.tile([128, K], BF16, tag="aO", name=f"aO{h}")
                    pv = pvp.tile([128, K], F32, tag="pv", name=f"pv{h}")
                    pts = []
                    for s in range(SCH):
                        st = pstp.tile([128, K], F32, tag="st",
                                       name=f"sT{h}_{s}")
                        for j in range(2):
                            js = slice(j * 512, (j + 1) * 512)
                            nc.tensor.matmul(
                                st[:, js], K_sb[h][:, s * 128:(s + 1) * 128],
                                QT_sb[h][:, js], start=True, stop=True)
                        if with_mask:
                            mrow = mrowp.tile([128, K], F32, tag="mrow")
                            nc.sync.dma_start(
                                mrow[:], mt_d[s * 128:(s + 1) * 128, :])
                            nc.vector.tensor_tensor(st[:], st[:], mrow[:],
                                                    ALU.add)
                        pT = probsp.tile([128, K], BF16, tag="pT",
                                         name=f"pT{h}_{s}")
                        nc.scalar.activation(pT[:], st[:], AF.Exp, scale=SCALE)
                        acc = aE if s % 2 == 0 else aO
                        if s < 2:
                            nc.vector.tensor_copy(acc[:], pT[:])
                        else:
                            nc.vector.tensor_tensor(acc[:], acc[:], pT[:],
                                                    ALU.add)
                        pts.append(pT)
                        if s == 2 and post_cb is not None:
                            post_cb()
                    for s in range(SCH):
                        for j in range(2):
                            js = slice(j * 512, (j + 1) * 512)
                            nc.tensor.matmul(
                                pv[:, js], V_sb[s][:, h * 128:(h + 1) * 128],
                                pts[s][:, js],
                                start=(s == 0), stop=(s == SCH - 1))
                    return (h, pv, aE, aO)

                state = None
                for h in range(NH):
                    cb = None
                    if state is not None:
                        cb = (lambda st=state: denom(*st))
                    state = attention(h, post_cb=cb)
                # last head's denominator: phase A finished a full phase-B
                # ago, so the chain runs entirely under the out-proj matmuls
                denom(*state)

                # transposed out-proj partial: for each 128-col chunk c of E,
                # outT[c][128 cols, 1024 tok] = sum_h wo_sb[h][:,c].T@attnT[h]
                # (PSUM-accumulated over the 4 local heads; h3 last so the
                # just-emitted denom(h3) chain hides under h0-h2 matmuls),
                # staged to bf16 (alternating DVE/ACT) and DMAd to DRAM.
                for c in range(E // 128):
                    ps = pstp.tile([128, K], F32, tag="st", name=f"ot{c}")
                    for h in range(NH):
                        for j in range(2):
                            js = slice(j * 512, (j + 1) * 512)
                            nc.tensor.matmul(
                                ps[:, js], wo_sb[h][:, c * 128:(c + 1) * 128],
                                attnT_sb[h][:, js],
                                start=(h == 0), stop=(h == NH - 1))
                    stg = ostgp.tile([128, K], BF16, tag="ostg")
                    if c % 2 == 0:
                        nc.vector.tensor_copy(stg[:], ps[:])
                    else:
                        nc.scalar.copy(stg[:], ps[:])
                    nc.sync.dma_start(partial_d[c * 128:(c + 1) * 128, :],
                                      stg[:])

                nc.gpsimd.collective_compute(
                    "ReduceScatter", ALU.add,
                    replica_groups=REPLICA_GROUPS,
                    ins=[partial_d[:].opt()],
                    outs=[rs_out_d[:].opt()],
                )
                nc.sync.dma_start(out_d[:], rs_out_d[:])

    return nc


def _split_multiwaits(nc):
    """walrus codegen in this container rejects instructions with more than
    one semaphore wait; split the excess onto preceding NoOps on the same
    engine."""
    for f in nc.m.functions:
        for blk in f.blocks:
            idx = 0
            while idx < len(blk.instructions):
                inst = blk.instructions[idx]
                si = inst.sync_info
                maxw = 1
                if si is None or len(si.on_wait) <= maxw:
                    idx += 1
                    continue
                waits = list(si.on_wait)
                ncarry = (len(waits) - 1) // maxw  # leave <=maxw on inst
                for k in range(ncarry):
                    chunk = waits[k * maxw:(k + 1) * maxw]
                    nop = mybir.InstNoOp(
                        name=nc.get_next_instruction_name(),
                        ins=[], outs=[],
                        bass_nofuse=True,
                        sync_info=mybir.SyncInfo(on_wait=chunk, on_update=[]),
                    )
                    nop.engine = inst.engine
                    nc.register_instruction(nop)
                    blk.instructions.insert(idx, nop)
                    idx += 1
                si.on_wait = waits[ncarry * maxw:]
                idx += 1


def _get_program(with_mask: bool):
    key = ("prog", with_mask)
    if key not in _CACHE:
        nc = _build(with_mask)
        _split_multiwaits(nc)
        _CACHE[key] = nc
    return _CACHE[key]


def kernel(x, context, attn_mask, w_q, w_k, w_v, w_ctx_k, w_ctx_v, w_out,
           q_norm_w, k_norm_w):
    x = np.asarray(x, np.float32)
    context = np.asarray(context, np.float32)
    attn_mask = np.asarray(attn_mask, np.float32)
    w_q = np.asarray(w_q, np.float32)
    w_k = np.asarray(w_k, np.float32)
    w_v = np.asarray(w_v, np.float32)
    w_ctx_k = np.asarray(w_ctx_k, np.float32)
    w_ctx_v = np.asarray(w_ctx_v, np.float32)
    w_out = np.asarray(w_out, np.float32)
    q_norm_w = np.asarray(q_norm_w, np.float32)
    k_norm_w = np.asarray(k_norm_w, np.float32)

    with_mask = bool(np.any(attn_mask))
    nc = _get_program(with_mask)
    in_maps = _prepare_in_maps(x, context, attn_mask, w_q, w_k, w_v, w_ctx_k,
                               w_ctx_v, w_out, q_norm_w, k_norm_w, with_mask)

    res = run_bass_kernel_spmd(nc, in_maps, list(range(NCORES))).results
    return _assemble(res)


def _assemble(res):
    out = np.empty((B, K, D), np.float32)
    for c in range(NCORES):
        b, g = c // GROUPS, c % GROUPS
        out[b, :, g * EW:(g + 1) * EW] = res[c]["out"].astype(np.float32).T
    return out


def _prepare_in_maps(x, context, attn_mask, w_q, w_k, w_v, w_ctx_k, w_ctx_v,
                     w_out, q_norm_w, k_norm_w, with_mask):
    bf16 = ml_dtypes.bfloat16
    xT = [np.ascontiguousarray(x[b].T).astype(bf16) for b in range(B)]
    cT = [np.ascontiguousarray(context[b].T).astype(bf16) for b in range(B)]
    in_maps = []
    for c in range(NCORES):
        b, g = c // GROUPS, c % GROUPS
        cols = slice(g * EW, (g + 1) * EW)
        m = {
            "xT": xT[b],
            "cT": cT[b],
            "wq": np.ascontiguousarray(w_q[:, cols]).astype(bf16),
            "wk": np.ascontiguousarray(w_k[:, cols]).astype(bf16),
            "wv": np.ascontiguousarray(w_v[:, cols]).astype(bf16),
            "wck": np.ascontiguousarray(w_ctx_k[:, cols]).astype(bf16),
            "wcv": np.ascontiguousarray(w_ctx_v[:, cols]).astype(bf16),
            "wo": np.ascontiguousarray(w_out[g * EW:(g + 1) * EW, :])
            .astype(bf16),
            "qnw": q_norm_w.reshape(HD, 1).astype(np.float32).copy(),
            "knw": k_norm_w.reshape(HD, 1).astype(np.float32).copy(),
        }
        if with_mask:
            # mask [B,1,K,S] -> transposed [S,K] per batch (fp32).
            # The kernel folds the 1/sqrt(HD) score scale into the exp
            # activation, which would scale the mask too; pre-divide so
            # exp((scores_raw + mask/SCALE) * SCALE) = exp(scores + mask).
            m["maskT"] = np.ascontiguousarray(attn_mask[b, 0].T) * (1.0 / SCALE)
        in_maps.append(m)
    return in_maps


# revision 28
# speedup vs baseline: 1.1021x; 1.0001x over previous
"""Trainium2 Bass kernel for DFlashAttentionV5.

Reference computation (fp32, single device):
    Q/K/V/Kctx/Vctx projections -> rmsnorm(Q), rmsnorm(K_full) -> softmax
    attention over concat(ctx, self) keys/values -> output projection.

Sharding over 8 NeuronCores: batch (2-way) x head-group (4-way).
Core c handles batch b = c // 4 and heads 4*g..4*g+3 where g = c % 4.
Each core computes attention for its 4 local heads, then the transposed
full-width output-projection partial outT[2048 cols, 1024 tokens] (same
PE cost as a 512-col slice: the contraction shrinks to the 512 local E
dims), and a single ReduceScatter sums the partials across the 4-core
batch group, leaving each core its 512-row slice of outT. The collective
cost model charges ~15us fixed + out_bytes/40GBps per op, so one 1MB-out
ReduceScatter (~41us) beats four chained 1MB-out AllGathers (~165us).

Pipeline notes (cost-model driven):
  - projections: rmsnorm of column-chunk c is emitted in two deferred
    stages during chunks c+1 / c+2 so its serial ACT/DVE chain never
    stalls the PE matmul stream.
  - attention per head: phase A emits scores+exp+denominator-accumulate
    for all 24 key-chunks, phase B the 24 PV matmuls (probs are kept in
    SBUF), so PSUM only needs 3 score slots + 1 PV slot; the softmax
    denominator accumulates in two interleaved bf16 SBUF accumulators
    (fast DVE mode, no serial-chain pacing) which are partition-reduced
    by PE ones-matmuls; each head's denominator->normalize chain is
    deferred into the next head's phase A.

All matmuls run in bf16 (fp32 PSUM accumulation); softmax statistics are
fp32. x/ctx arrive pre-transposed from the host ([D, tokens]) so no
on-device transposes are needed.

Self-contained: hardcodes all shapes; only imports concourse + numpy.
"""

import math

import numpy as np
import ml_dtypes

import concourse.bass as bass
import concourse.mybir as mybir
import concourse.tile as tile
from concourse.bass_utils import run_bass_kernel_spmd

BF16 = mybir.dt.bfloat16
F32 = mybir.dt.float32
AF = mybir.ActivationFunctionType
ALU = mybir.AluOpType

# Problem dims
B, K, CTX, D, H, HD = 2, 1024, 2048, 2048, 16, 128
S = CTX + K            # 3072 keys per query
NCORES = 8
GROUPS = 4             # head groups (tensor-parallel within a batch)
NH = H // GROUPS       # 4 local heads per core
E = H * HD             # 2048
EW = NH * HD           # 512 local attention width / weight shard width
DCH = D // 128         # 16 contraction chunks
SCH = S // 128         # 24 key chunks
TCH = K // 128         # 8 query-token chunks
SCALE = 1.0 / math.sqrt(HD)
EPS = 1e-6
REPLICA_GROUPS = [[0, 1, 2, 3], [4, 5, 6, 7]]

_CACHE = {}


def _build(with_mask: bool):
    """Build the SPMD bass program (same program on all 8 cores)."""
    nc = bass.Bass(num_devices=NCORES)

    xT_d = nc.declare_dram_parameter("xT", [D, K], BF16, isOutput=False)
    cT_d = nc.declare_dram_parameter("cT", [D, CTX], BF16, isOutput=False)
    wq_d = nc.declare_dram_parameter("wq", [D, EW], BF16, isOutput=False)
    wk_d = nc.declare_dram_parameter("wk", [D, EW], BF16, isOutput=False)
    wv_d = nc.declare_dram_parameter("wv", [D, EW], BF16, isOutput=False)
    wck_d = nc.declare_dram_parameter("wck", [D, EW], BF16, isOutput=False)
    wcv_d = nc.declare_dram_parameter("wcv", [D, EW], BF16, isOutput=False)
    wo_d = nc.declare_dram_parameter("wo", [EW, E], BF16, isOutput=False)
    qnw_d = nc.declare_dram_parameter("qnw", [HD, 1], F32, isOutput=False)
    knw_d = nc.declare_dram_parameter("knw", [HD, 1], F32, isOutput=False)
    if with_mask:
        mt_d = nc.declare_dram_parameter("maskT", [S, K], F32, isOutput=False)
    # transposed output slice [out-cols 512, tokens 1024], host transposes
    out_d = nc.declare_dram_parameter("out", [EW, K], BF16, isOutput=True)

    with tile.TileContext(nc, num_cores=NCORES) as tc:
        with (
            tc.tile_pool(name="const", bufs=1) as constp,
            tc.tile_pool(name="perm", bufs=1) as perm,
            tc.tile_pool(name="stat", bufs=2) as statp,
            tc.tile_pool(name="bc", bufs=2) as bcp,
            tc.tile_pool(name="dram", bufs=1, space="DRAM") as dram,
        ):
            ones_col = constp.tile([128, 1], BF16)
            nc.any.memset(ones_col, 1.0)
            ones_row = constp.tile([1, 128], BF16)
            nc.any.memset(ones_row, 1.0)
            qnw_sb = constp.tile([HD, 1], F32)
            knw_sb = constp.tile([HD, 1], F32)

            # Resident tensors (bf16):
            #   K_sb[h]  [128=hd, 3072=s] per local head (ctx keys then self)
            #   V_sb[s]  [128=s-chunk, 512=4 heads x hd], s 0..15 ctx, 16..23 self
            #   QT_sb[h] [128=hd, 1024=q]
            K_sb = [perm.tile([128, S], BF16, tag=f"K{h}", bufs=1, name=f"K{h}")
                    for h in range(NH)]
            V_sb = [perm.tile([128, EW], BF16, tag=f"V{s}", bufs=1, name=f"V{s}")
                    for s in range(SCH)]
            QT_sb = [perm.tile([128, K], BF16, tag=f"Q{h}", bufs=1, name=f"Q{h}")
                     for h in range(NH)]
            attnT_sb = [perm.tile([128, K], BF16, tag=f"A{h}", bufs=1,
                                  name=f"A{h}") for h in range(NH)]

            # full-width transposed out-proj partial, reduce-scattered
            partial_d = dram.tile([E, K], BF16, name="partial")
            rs_out_d = dram.tile([EW, K], BF16, name="rsout")

            # ================= projection phase =================
            with (
                tc.tile_pool(name="srcT", bufs=1) as srcTp,
                tc.tile_pool(name="wstream", bufs=4) as wstream,
                tc.tile_pool(name="wwide", bufs=1) as wwide,
                tc.tile_pool(name="sqp", bufs=2) as sqp,
                tc.tile_pool(name="psA", bufs=3, space="PSUM") as psA,
                tc.tile_pool(name="ps1", bufs=2, space="PSUM") as ps1,
            ):
                # ---- rmsnorm in transposed layout, software-pipelined.
                # ps [128=hd, 1024=tokens] fp32 PSUM -> dest bf16 SBUF.
                # stage0 (immediate): square on ACT.
                # stage1 (next chunk): mean via ones-matmul, +eps, recip,
                #   rsqrt (all short ops off the PE except one tiny matmul).
                # stage2 (chunk after): broadcast matmul, copy, apply.
                def rms_stage0(ps):
                    # square for the mean, plus a bf16 snapshot of the
                    # projection values so stage2 reads SBUF and the PSUM
                    # tile retires immediately (keeps the psA ring free)
                    sqt = sqp.tile([128, 1024], BF16, tag="sq")
                    nc.scalar.square(sqt[:], ps[:])
                    psb = sqp.tile([128, 1024], BF16, tag="psb", bufs=3)
                    nc.vector.tensor_copy(psb[:], ps[:])
                    return sqt, psb

                def rms_stage1(sqt):
                    rss = []
                    for j in range(2):
                        js = slice(j * 512, (j + 1) * 512)
                        ps_s = ps1.tile([128, 512], F32, tag="ps1")
                        nc.tensor.matmul(ps_s[0:1, :], ones_col[:], sqt[:, js],
                                         start=True, stop=True)
                        mean = statp.tile([1, 512], F32, tag="mean")
                        nc.vector.tensor_scalar(mean[:], ps_s[0:1, :],
                                                1.0 / HD, EPS,
                                                ALU.mult, ALU.add)
                        rec = statp.tile([1, 512], F32, tag="rec")
                        nc.vector.reciprocal(rec[:], mean[:])
                        rs = statp.tile([1, 512], BF16, tag="rs")
                        nc.scalar.sqrt(rs[:], rec[:])  # rsqrt = sqrt(1/x)
                        rss.append(rs)
                    return rss

                def rms_stage2(psb, rss, dest_ap, nw_sb):
                    for j in range(2):
                        js = slice(j * 512, (j + 1) * 512)
                        ps_b = ps1.tile([128, 512], F32, tag="ps1")
                        nc.tensor.matmul(ps_b[:], ones_row[:], rss[j][:],
                                         start=True, stop=True)
                        bc = bcp.tile([128, 512], F32, tag="bc")
                        nc.scalar.copy(bc[:], ps_b[:])
                        nc.vector.scalar_tensor_tensor(
                            dest_ap[:, js], psb[:, js], nw_sb[:], bc[:],
                            ALU.mult, ALU.mult)

                # deferred-emission pipeline: stage1(c) emits during chunk
                # c+1, stage2(c) during chunk c+2, so the serial rms chain
                # always has a full chunk of matmuls to hide behind.
                pend1 = []  # stage1 closures; popped -> emit, yields stage2
                pend2 = []  # stage2 closures awaiting emission

                def boundary():
                    # stage1(c-1) first: its tiny PE ops depend only on the
                    # long-finished square; stage2(c-2)'s broadcast matmul
                    # then finds its rsqrt row already computed.
                    new2 = pend1.pop(0)() if pend1 else None
                    if pend2:
                        pend2.pop(0)()
                    if new2 is not None:
                        pend2.append(new2)

                def flush():
                    while pend1 or pend2:
                        boundary()

                # d-chunk accessor over grouped source tiles [128, 4*1024]
                def src_at(grp, d):
                    return grp[d // 4], (d % 4) * 1024

                def load_set(grp, dram_ap, split_first=False):
                    # dram_ap: [D, 1024] (d-major); one DMA per 4 d-chunks.
                    # split_first peels d-chunk 0 into its own small DMA so
                    # the first dependent matmul can start sooner.
                    if split_first:
                        nc.sync.dma_start(
                            grp[0][:, 0:1024],
                            dram_ap[0:128, :])
                        nc.sync.dma_start(
                            grp[0][:, 1024:4096].rearrange("p (a t) -> p a t",
                                                           t=1024),
                            dram_ap[128:512, :]
                            .rearrange("(a p) t -> p a t", p=128))
                    else:
                        nc.sync.dma_start(
                            grp[0][:].rearrange("p (a t) -> p a t", t=1024),
                            dram_ap[0:512, :]
                            .rearrange("(a p) t -> p a t", p=128))
                    for i in range(1, 4):
                        nc.sync.dma_start(
                            grp[i][:].rearrange("p (a t) -> p a t", t=1024),
                            dram_ap[i * 512:(i + 1) * 512, :]
                            .rearrange("(a p) t -> p a t", p=128))

                # Q^T / K^T projections (weight-stationary):
                # psum[c] [128=col-chunk(head), 1024 tokens] += w[d,c].T @ srcT[d]
                def load_wchunk(w_d, c, name=None):
                    wch = wstream.tile([128, D], BF16, tag="w", name=name)
                    nc.sync.dma_start(
                        wch[:].rearrange("p (a q) -> p a q", q=128),
                        w_d[:, c * 128:(c + 1) * 128]
                        .rearrange("(a p) q -> p a q", p=128))
                    return wch

                def qk_proj(w_d, srcT, dest_of_chunk, nw_sb, pre=None):
                    for c in range(EW // 128):
                        wch = pre if (c == 0 and pre is not None) \
                            else load_wchunk(w_d, c)
                        ps = psA.tile([128, 1024], F32, tag="psA",
                                      name=f"psqk{c}")
                        for d in range(DCH):
                            st, off = src_at(srcT, d)
                            for j in range(2):
                                nc.tensor.matmul(
                                    ps[:, j * 512:(j + 1) * 512],
                                    wch[:, d * 128:(d + 1) * 128],
                                    st[:, off + j * 512:off + j * 512 + 512],
                                    start=(d == 0), stop=(d == DCH - 1))
                        boundary()
                        sqt, psb = rms_stage0(ps)
                        dest, off = dest_of_chunk(c)

                        def s1(sqt=sqt, psb=psb, dest=dest, off=off,
                               nw=nw_sb):
                            rss = rms_stage1(sqt)
                            return lambda: rms_stage2(
                                psb, rss, dest[:, off:off + 1024], nw)

                        pend1.append(s1)

                # V projections (activation-stationary):
                # V_sb[s] [128=tokens, 512=cols] += srcT[d][:,t-chunk].T @ wv[d]
                def load_wide(w_d, tag):
                    grp = []
                    for i in range(4):
                        wt = wwide.tile([128, 4 * EW], BF16, tag=f"wv{i}",
                                        bufs=1, name=f"wv{i}_{tag}")
                        nc.sync.dma_start(
                            wt[:].rearrange("p (a q) -> p a q", q=EW),
                            w_d[i * 512:(i + 1) * 512, :]
                            .rearrange("(a p) q -> p a q", p=128))
                        grp.append(wt)
                    return grp

                def v_proj(wv_grp, srcT, s_base):
                    for t in range(TCH):
                        ps = ps1.tile([128, 512], F32, tag="ps1")
                        for d in range(DCH):
                            st, off = src_at(srcT, d)
                            wvt = wv_grp[d // 4]
                            wo_off = (d % 4) * EW
                            nc.tensor.matmul(
                                ps[:], st[:, off + t * 128:off + (t + 1) * 128],
                                wvt[:, wo_off:wo_off + EW],
                                start=(d == 0), stop=(d == DCH - 1))
                        nc.vector.tensor_copy(V_sb[s_base + t][:], ps[:])
                        boundary()

                # slot sets: A = xT then ctx-half1 (recycled), B = ctx-half0
                setA = [srcTp.tile([128, 4096], BF16, tag=f"sa{i}", bufs=1,
                                   name=f"xT{i}") for i in range(4)]
                setB = [srcTp.tile([128, 4096], BF16, tag=f"sb{i}", bufs=1,
                                   name=f"cTa{i}") for i in range(4)]
                # first weight chunk: peel d-chunks 0-1 into a small DMA so
                # the very first matmul only waits ~0.5us of DMA
                pre_wq = wstream.tile([128, D], BF16, tag="w", name="prewq")
                nc.sync.dma_start(
                    pre_wq[:, 0:256].rearrange("p (a q) -> p a q", q=128),
                    wq_d[0:256, 0:128].rearrange("(a p) q -> p a q", p=128))
                nc.sync.dma_start(
                    pre_wq[:, 256:D].rearrange("p (a q) -> p a q", q=128),
                    wq_d[256:D, 0:128].rearrange("(a p) q -> p a q", p=128))
                load_set(setA, xT_d, split_first=True)
                nc.sync.dma_start(qnw_sb[:], qnw_d[:])
                nc.sync.dma_start(knw_sb[:], knw_d[:])

                # self tokens (block A): Q, K_self, V_self
                # (V weights queued before the big setB source DMA so
                # v_proj(setA) doesn't stall behind it)
                qk_proj(wq_d, setA, lambda c: (QT_sb[c], 0), qnw_sb, pre=pre_wq)
                wv_sb = load_wide(wv_d, "s")
                load_set(setB, cT_d[:, 0:1024])
                qk_proj(wk_d, setA, lambda c: (K_sb[c], CTX), knw_sb)
                v_proj(wv_sb, setA, CTX // 128)

                # ctx half 0 (block B): K_ctx[:, 0:1024], V_ctx s-chunks 0..7
                wcv_sb = load_wide(wcv_d, "c0")
                qk_proj(wck_d, setB, lambda c: (K_sb[c], 0), knw_sb)
                v_proj(wcv_sb, setB, 0)

                # ctx half 1 reuses set A slots
                setC = [srcTp.tile([128, 4096], BF16, tag=f"sa{i}", bufs=1,
                                   name=f"cTb{i}") for i in range(4)]
                load_set(setC, cT_d[:, 1024:2048])
                wcv2_sb = load_wide(wcv_d, "c1")
                qk_proj(wck_d, setC, lambda c: (K_sb[c], 1024), knw_sb)
                v_proj(wcv2_sb, setC, TCH)
                flush()

            # ================= attention + output phase =================
            with (
                tc.tile_pool(name="probsT", bufs=24) as probsp,
                tc.tile_pool(name="acc", bufs=2) as accp,
                tc.tile_pool(name="wop", bufs=1) as wop,
                tc.tile_pool(name="ostg", bufs=3) as ostgp,
                tc.tile_pool(name="mrow", bufs=4) as mrowp,
                tc.tile_pool(name="pst", bufs=3, space="PSUM") as pstp,
                tc.tile_pool(name="pvp", bufs=1, space="PSUM") as pvp,
            ):
                # w_out rows for local head h (global rows (g*NH+h)*128..),
                # all 2048 columns: [128=hd, 2048=out-cols]
                wo_sb = [wop.tile([128, E], BF16, tag=f"wo{h}", bufs=1,
                                  name=f"wo{h}") for h in range(NH)]
                for h in range(NH):
                    nc.sync.dma_start(wo_sb[h][:],
                                      wo_d[h * 128:(h + 1) * 128, :])

                # softmax denominator + normalize for one finished head:
                # column sums of the two bf16 accumulators via PE ones-
                # matmuls (PSUM-accumulated), reciprocal, broadcast across
                # partitions via K=1 matmul, then attnT = pv * bcast(1/den).
                def denom(h, pv, aE, aO):
                    ps_d = pstp.tile([1, K], F32, tag="st", name=f"psd{h}")
                    for j in range(2):
                        js = slice(j * 512, (j + 1) * 512)
                        nc.tensor.matmul(ps_d[0:1, js], ones_col[:],
                                         aE[:, js], start=True, stop=False)
                        nc.tensor.matmul(ps_d[0:1, js], ones_col[:],
                                         aO[:, js], start=False, stop=True)
                    rec = statp.tile([1, K], F32, tag="recA")
                    nc.vector.reciprocal(rec[:], ps_d[0:1, :])
                    rb = statp.tile([1, K], BF16, tag="rbA")
                    nc.vector.tensor_copy(rb[:], rec[:])
                    ps_b = pstp.tile([128, K], F32, tag="st", name=f"psb{h}")
                    for j in range(2):
                        js = slice(j * 512, (j + 1) * 512)
                        nc.tensor.matmul(ps_b[:, js], ones_row[:],
                                         rb[0:1, js], start=True, stop=True)
                    bc = bcp.tile([128, K], F32, tag="bcA")
                    nc.scalar.copy(bc[:], ps_b[:])
                    for j in range(2):
                        js = slice(j * 512, (j + 1) * 512)
                        nc.vector.tensor_tensor(attnT_sb[h][:, js],
                                                pv[:, js], bc[:, js],
                                                ALU.mult)

                # attention for one local head, transposed scores:
                # scoresT[s-chunk] [128=s, 1024=q] = K_chunk @ Q^T  (no max
                # subtraction: scores ~ N(0,1) after rmsnorm + 1/sqrt(HD)).
                # Phase A: scores+exp+acc per key chunk; probs stay in SBUF.
                # Phase B: the 24 PV accumulation matmuls.
                def attention(h, post_cb=None):
                    aE = accp.tile([128, K], BF16, tag="aE", name=f"aE{h}")
                    aO = accp.tile([128, K], BF16, tag="aO", name=f"aO{h}")
                    pv = pvp.tile([128, K], F32, tag="pv", name=f"pv{h}")
                    pts = []
                    for s in range(SCH):
                        st = pstp.tile([128, K], F32, tag="st",
                                       name=f"sT{h}_{s}")
                        for j in range(2):
                            js = slice(j * 512, (j + 1) * 512)
                            nc.tensor.matmul(
                                st[:, js], K_sb[h][:, s * 128:(s + 1) * 128],
                                QT_sb[h][:, js], start=True, stop=True)
                        if with_mask:
                            mrow = mrowp.tile([128, K], F32, tag="mrow")
                            nc.sync.dma_start(
                                mrow[:], mt_d[s * 128:(s + 1) * 128, :])
                            nc.vector.tensor_tensor(st[:], st[:], mrow[:],
                                                    ALU.add)
                        pT = probsp.tile([128, K], BF16, tag="pT",
                                         name=f"pT{h}_{s}")
                        nc.scalar.activation(pT[:], st[:], AF.Exp, scale=SCALE)
                        acc = aE if s % 2 == 0 else aO
                        if s < 2:
                            nc.vector.tensor_copy(acc[:], pT[:])
                        else:
                            nc.vector.tensor_tensor(acc[:], acc[:], pT[:],
                                                    ALU.add)
                        pts.append(pT)
                        if s == 2 and post_cb is not None:
                            post_cb()
                    for s in range(SCH):
                        for j in range(2):
                            js = slice(j * 512, (j + 1) * 512)
                            nc.tensor.matmul(
                                pv[:, js], V_sb[s][:, h * 128:(h + 1) * 128],
                                pts[s][:, js],
                                start=(s == 0), stop=(s == SCH - 1))
                    return (h, pv, aE, aO)

                state = None
                for h in range(NH):
                    cb = None
                    if state is not None:
                        cb = (lambda st=state: denom(*st))
                    state = attention(h, post_cb=cb)
                # last head's denominator: phase A finished a full phase-B
                # ago, so the chain runs entirely under the out-proj matmuls
                denom(*state)

                # transposed out-proj partial: for each 128-col chunk c of E,
                # outT[c][128 cols, 1024 tok] = sum_h wo_sb[h][:,c].T@attnT[h]
                # (PSUM-accumulated over the 4 local heads; h3 last so the
                # just-emitted denom(h3) chain hides under h0-h2 matmuls),
                # staged to bf16 (alternating DVE/ACT) and DMAd to DRAM.
                for c in range(E // 128):
                    ps = pstp.tile([128, K], F32, tag="st", name=f"ot{c}")
                    for h in range(NH):
                        for j in range(2):
                            js = slice(j * 512, (j + 1) * 512)
                            nc.tensor.matmul(
                                ps[:, js], wo_sb[h][:, c * 128:(c + 1) * 128],
                                attnT_sb[h][:, js],
                                start=(h == 0), stop=(h == NH - 1))
                    stg = ostgp.tile([128, K], BF16, tag="ostg")
                    if c % 2 == 0:
                        nc.vector.tensor_copy(stg[:], ps[:])
                    else:
                        nc.scalar.copy(stg[:], ps[:])
                    nc.sync.dma_start(partial_d[c * 128:(c + 1) * 128, :],
                                      stg[:])

                nc.gpsimd.collective_compute(
                    "ReduceScatter", ALU.add,
                    replica_groups=REPLICA_GROUPS,
                    ins=[partial_d[:].opt()],
                    outs=[rs_out_d[:].opt()],
                )
                nc.sync.dma_start(out_d[:], rs_out_d[:])

    return nc


def _split_multiwaits(nc):
    """walrus codegen in this container rejects instructions with more than
    one semaphore wait; split the excess onto preceding NoOps on the same
    engine."""
    for f in nc.m.functions:
        for blk in f.blocks:
            idx = 0
            while idx < len(blk.instructions):
                inst = blk.instructions[idx]
                si = inst.sync_info
                maxw = 1
                if si is None or len(si.on_wait) <= maxw:
                    idx += 1
                    continue
                waits = list(si.on_wait)
                ncarry = (len(waits) - 1) // maxw  # leave <=maxw on inst
                for k in range(ncarry):
                    chunk = waits[k * maxw:(k + 1) * maxw]
                    nop = mybir.InstNoOp(
                        name=nc.get_next_instruction_name(),
                        ins=[], outs=[],
                        bass_nofuse=True,
                        sync_info=mybir.SyncInfo(on_wait=chunk, on_update=[]),
                    )
                    nop.engine = inst.engine
                    nc.register_instruction(nop)
                    blk.instructions.insert(idx, nop)
                    idx += 1
                si.on_wait = waits[ncarry * maxw:]
                idx += 1


def _get_program(with_mask: bool):
    key = ("prog", with_mask)
    if key not in _CACHE:
        nc = _build(with_mask)
        _split_multiwaits(nc)
        _CACHE[key] = nc
    return _CACHE[key]


def kernel(x, context, attn_mask, w_q, w_k, w_v, w_ctx_k, w_ctx_v, w_out,
           q_norm_w, k_norm_w):
    x = np.asarray(x, np.float32)
    context = np.asarray(context, np.float32)
    attn_mask = np.asarray(attn_mask, np.float32)
    w_q = np.asarray(w_q, np.float32)
    w_k = np.asarray(w_k, np.float32)
    w_v = np.asarray(w_v, np.float32)
    w_ctx_k = np.asarray(w_ctx_k, np.float32)
    w_ctx_v = np.asarray(w_ctx_v, np.float32)
    w_out = np.asarray(w_out, np.float32)
    q_norm_w = np.asarray(q_norm_w, np.float32)
    k_norm_w = np.asarray(k_norm_w, np.float32)

    with_mask = bool(np.any(attn_mask))
    nc = _get_program(with_mask)
    in_maps = _prepare_in_maps(x, context, attn_mask, w_q, w_k, w_v, w_ctx_k,
                               w_ctx_v, w_out, q_norm_w, k_norm_w, with_mask)

    res = run_bass_kernel_spmd(nc, in_maps, list(range(NCORES))).results
    return _assemble(res)


def _assemble(res):
    out = np.empty((B, K, D), np.float32)
    for c in range(NCORES):
        b, g = c // GROUPS, c % GROUPS
        out[b, :, g * EW:(g + 1) * EW] = res[c]["out"].astype(np.float32).T
    return out


def _prepare_in_maps(x, context, attn_mask, w_q, w_k, w_v, w_ctx_k, w_ctx_v,
                     w_out, q_norm_w, k_norm_w, with_mask):
    bf16 = ml_dtypes.bfloat16
    xT = [np.ascontiguousarray(x[b].T).astype(bf16) for b in range(B)]
    cT = [np.ascontiguousarray(context[b].T).astype(bf16) for b in range(B)]
    in_maps = []
    for c in range(NCORES):
        b, g = c // GROUPS, c % GROUPS
        cols = slice(g * EW, (g + 1) * EW)
        m = {
            "xT": xT[b],
            "cT": cT[b],
            "wq": np.ascontiguousarray(w_q[:, cols]).astype(bf16),
            "wk": np.ascontiguousarray(w_k[:, cols]).astype(bf16),
            "wv": np.ascontiguousarray(w_v[:, cols]).astype(bf16),
            "wck": np.ascontiguousarray(w_ctx_k[:, cols]).astype(bf16),
            "wcv": np.ascontiguousarray(w_ctx_v[:, cols]).astype(bf16),
            "wo": np.ascontiguousarray(w_out[g * EW:(g + 1) * EW, :])
            .astype(bf16),
            "qnw": q_norm_w.reshape(HD, 1).astype(np.float32).copy(),
            "knw": k_norm_w.reshape(HD, 1).astype(np.float32).copy(),
        }
        if with_mask:
            # mask [B,1,K,S] -> transposed [S,K] per batch (fp32).
            # The kernel folds the 1/sqrt(HD) score scale into the exp
            # activation, which would scale the mask too; pre-divide so
            # exp((scores_raw + mask/SCALE) * SCALE) = exp(scores + mask).
            m["maskT"] = np.ascontiguousarray(attn_mask[b, 0].T) * (1.0 / SCALE)
        in_maps.append(m)
    return in_maps
